# revision 32
# baseline (speedup 1.0000x reference)
"""Causal self-attention (B=2, T=4096, C=768, H=12) on 8 TRN2 NeuronCores.

Sharding: batch x head-group. Core c handles batch b=c//4 and heads
h0..h0+2 where h0 = 3*(c%4). Each core computes the qkv projection for
its 3 heads, full causal attention, and a partial output projection; the
host sums the 4 partials per batch and adds the (v-bias-folded)
projection bias.

All matmul operands are bf16 (1 cycle/row on the PE at any moving size).
q/k live transposed ([D, T]) feeding the scores matmul; v is computed in
natural token-major layout and packed into v_aug [k, 65*3] with a ones
column per head so the softmax denominator falls out of att@v as output
column 64. att@v runs in [q, d] orientation (stationary eb^T chunk
[k,128q], moving v_aug [k,65]) so each 128x128 block costs only 65 PE
cycles and the denominator lands as a per-partition column, normalized
with reciprocal + tensor_scalar (no partition broadcast). The normalized
ao [tok, hd] is flipped to [hd, tok] with DMA-engine xbar transposes and
fed to the output projection as the stationary operand.

Causality: scores/exp are only computed for k-tiles at or below the
diagonal, trimmed to the valid q-range on the diagonal band; the
remaining intra-block mask is applied in-place on eb by gpsimd
affine_select. The v bias is algebraically folded into the host-side
projection bias (softmax weights sum to 1).
"""

import sys

for _p in ("/opt/trn_rl_repo",):
    if _p not in sys.path:
        sys.path.insert(0, _p)

from contextlib import ExitStack

import numpy as np

import concourse.bass as bass  # noqa: F401
import concourse.mybir as mybir
import concourse.tile as tile
from concourse import bacc
from concourse.bass_utils import run_bass_kernel_spmd

f32 = mybir.dt.float32
bf16 = mybir.dt.bfloat16
AF = mybir.ActivationFunctionType

C = 768
D = 64
N_HEAD = 12
HPC = 3  # heads per core
N_CORES = 8

# wq column slots: q01 | k01 | (q2 stacked over k2)
QK_SLOTS = [(0, 128), (128, 256), (256, 384)]


def build_nc(T):
    NT = T // 512  # q tiles
    KT = T // 128  # k tiles / token chunks
    CK = C // 128  # contraction chunks

    nc = bacc.Bacc("TRN2", target_bir_lowering=False, debug=False,
                   num_devices=N_CORES)
    xt_d = nc.dram_tensor("xt", [C, T], bf16, kind="ExternalInput").ap()
    wq_d = nc.dram_tensor("wq", [C, 384], bf16, kind="ExternalInput").ap()
    wv_d = nc.dram_tensor("wv", [C, HPC * D], bf16, kind="ExternalInput").ap()
    bq_d = nc.dram_tensor("bq", [128, 3], f32, kind="ExternalInput").ap()
    wp_d = nc.dram_tensor("wp", [HPC * D, C], bf16, kind="ExternalInput").ap()
    y_d = nc.dram_tensor("y", [T, C], f32, kind="ExternalOutput").ap()
    import os
    dbg = os.environ.get("KDBG") == "1"
    dbg_out = {}
    if dbg:
        for nm, shp, dt in [("d_qAB", [128, T], bf16), ("d_kAB", [128, T], bf16),
                            ("d_qC", [128, T], bf16), ("d_kC", [128, T], bf16),
                            ("d_vaug", [128, 32 * 195], bf16),
                            ("d_eb", [128, 1024], bf16),
                            ("d_accs", [128, 780], f32),
                            ("d_ao", [128, 192], bf16),
                            ("d_aotab", [128, 128], bf16),
                            ("d_aotbc", [128, 128], bf16)]:
            dbg_out[nm] = nc.dram_tensor(nm, shp, dt, kind="ExternalOutput").ap()

    with tile.TileContext(nc) as tc, ExitStack() as ctx:
        sb = ctx.enter_context(tc.tile_pool(name="sb", bufs=1))

        # persistent tensors
        bq_sb = sb.tile([128, 3], f32, tag="bq")
        qT_AB = sb.tile([128, T], bf16, tag="qAB")
        kT_AB = sb.tile([128, T], bf16, tag="kAB")
        qT_C = sb.tile([128, T], bf16, tag="qC")
        kT_C = sb.tile([128, T], bf16, tag="kC")
        v_aug = sb.tile([128, KT * 195], bf16, tag="vaug")
        wq_sb = [sb.tile([128, 384], bf16, tag=f"wq{c}", name=f"wq{c}")
                 for c in range(CK)]
        wv_sb = [sb.tile([128, HPC * D], bf16, tag=f"wv{c}", name=f"wv{c}")
                 for c in range(CK)]
        wpA_sb = sb.tile([128, C], bf16, tag="wpA")   # heads 0,1
        # head 2 lives at partitions 64:128 to match aoT_bc's h2 strip
        wpC_sb = sb.tile([128, C], bf16, tag="wpC")

        nc.sync.dma_start(bq_sb[:], bq_d)
        # ones columns of v_aug (cols 64,129,194 mod 195) come from this
        # blanket fill; the v copies below overwrite the 64-col data slices.
        nc.gpsimd.memset(v_aug[:], 1.0)

        # PE warm-up: ~3.5us of dummy matmuls bridge the initial xt DMA
        # latency so the p-state ramp completes before the first real matmul
        warm = sb.tile([128, 512], bf16, tag="warm")
        nc.vector.memset(warm[:], 0.5)
        with tc.tile_pool(name="wups", bufs=1, space="PSUM") as wu_ps:
            wu = wu_ps.tile([128, 512], f32, tag="wu")
            for _ in range(12):
                nc.tensor.matmul(wu[:], warm[:, 0:128], warm[:],
                                 start=True, stop=True,
                                 skip_group_check=True)

        # xt strips: whole [128, T] rows per contraction chunk; the j=0
        # slice loads first so the pipeline can start early.
        xt_sb = [sb.tile([128, T], bf16, tag=f"xts{c}", name=f"xts{c}")
                 for c in range(CK)]
        for c in range(CK):
            nc.sync.dma_start(wq_sb[c][:], wq_d[c * 128:(c + 1) * 128, :])
            nc.sync.dma_start(xt_sb[c][:, 0:1024],
                              xt_d[c * 128:(c + 1) * 128, 0:1024])
        for c in range(CK):
            nc.sync.dma_start(wv_sb[c][:], wv_d[c * 128:(c + 1) * 128, :])
        nc.sync.dma_start(wpA_sb[:], wp_d[0:128, :])
        nc.sync.dma_start(wpC_sb[64:128, :], wp_d[128:192, :])
        for half in range(1, 4):
            hsl = slice(half * 1024, (half + 1) * 1024)
            for c in range(CK):
                nc.sync.dma_start(xt_sb[c][:, hsl],
                                  xt_d[c * 128:(c + 1) * 128, hsl])
        # PSUM budget (8 banks): sc 2x2 + acc 2 + qkv 2. The acc slots
        # also serve the deferred projection (they are idle between norm(j)
        # and attv(j+1)); qkv rotates its 5 generations through 2 banks.
        sc_ps = ctx.enter_context(
            tc.tile_pool(name="scps", bufs=2, space="PSUM"))
        at_ps = ctx.enter_context(
            tc.tile_pool(name="atps", bufs=2, space="PSUM"))
        qk_ps = ctx.enter_context(
            tc.tile_pool(name="qkps", bufs=2, space="PSUM"))
        eb_pool = ctx.enter_context(tc.tile_pool(name="ebp", bufs=12))
        ao_pool = ctx.enter_context(tc.tile_pool(name="aop", bufs=6))
        aoT_pool = ctx.enter_context(tc.tile_pool(name="aotp", bufs=6))
        y_pool = ctx.enter_context(tc.tile_pool(name="yp", bufs=4))
        nrm = ctx.enter_context(tc.tile_pool(name="nrm", bufs=4))

        def qkv_tasks(j):
            """qkv projection for q-tile j as a list of filler closures (one
            per psum generation) so the PE work can interleave between
            attention rounds of the previous tile."""
            jsl = bass.ts(j, 512)
            xt_t = [xt_sb[c][:, jsl] for c in range(CK)]

            def qk_slot(ps_reg, s):
                c0, c1 = QK_SLOTS[s]
                for c in range(CK):
                    nc.tensor.matmul(ps_reg, wq_sb[c][:, c0:c1], xt_t[c],
                                     start=(c == 0), stop=(c == CK - 1),
                                     skip_group_check=True)

            def v_chunk(ps_reg, tc_):
                for c in range(CK):
                    nc.tensor.matmul(ps_reg,
                                     xt_t[c][:, tc_ * 128:(tc_ + 1) * 128],
                                     wv_sb[c][:],
                                     start=(c == 0), stop=(c == CK - 1),
                                     skip_group_check=True)

            def v_store(ps_reg, tc_):
                base = (4 * j + tc_) * 195
                dst = v_aug[:, base:base + 195].rearrange(
                    "p (h c) -> p h c", c=65)[:, :, 0:64]
                nc.vector.tensor_copy(
                    dst, ps_reg.rearrange("p (h c) -> p h c", c=64))

            def gen1():
                g = qk_ps.tile([128, 512], f32, tag="qk", name="qkv1")
                qk_slot(g[:], 0)
                nc.vector.tensor_scalar_add(qT_AB[:, jsl], g[:],
                                            bq_sb[:, 0:1])

            def gen2():
                g = qk_ps.tile([128, 512], f32, tag="qk", name="qkv2")
                qk_slot(g[:], 1)
                nc.vector.tensor_scalar_add(kT_AB[:, jsl], g[:],
                                            bq_sb[:, 1:2])

            def gen3():
                g = qk_ps.tile([128, 512], f32, tag="qk", name="qkv3")
                qk_slot(g[:], 2)
                # q2 rows 0:64, k2 rows 64:128 stacked in one bank
                nc.vector.tensor_scalar_add(qT_C[0:64, jsl], g[0:64, :],
                                            bq_sb[0:64, 2:3])
                nc.vector.tensor_scalar_add(kT_C[64:128, jsl], g[64:128, :],
                                            bq_sb[64:128, 2:3])
                # duplicate head-2 q/k into the other 64-partition strip so
                # the scores matmul can alternate strips (operands must share
                # the partition range)
                nc.sync.dma_start(qT_C[64:128, jsl], qT_C[0:64, jsl])
                nc.sync.dma_start(kT_C[0:64, jsl], kT_C[64:128, jsl])

            def mkv(tc_):
                # one accumulation group per psum generation: a start=True
                # in a bank wipes other in-flight groups' pending state
                def gen():
                    g = qk_ps.tile([128, 512], f32, tag="qk",
                                   name=f"qkv4_{tc_}")
                    v_chunk(g[:, 0:192], tc_)
                    v_store(g[:, 0:192], tc_)
                return gen

            # (must_before_tile, cost, closure): gen1/2/3 must run early
            # enough that tile j's scores never wait on them; the v chunks
            # by the start of attention(j)
            gens = [gen1, gen2, gen3] + [mkv(t) for t in range(4)]
            tags = [j - 0.4] * 3 + [j] * 4
            costs = [1280] * 3 + [480] * 4
            return list(zip(tags, costs, gens))

        import os
        KSTAGE = int(os.environ.get("KSTAGE", "5"))

        # chunks whose normalize+transpose ran in iteration j; their output
        # projection is deferred into iteration j+1 so the PE never waits on
        # a transpose DMA at the head of its in-order queue.
        pending_proj = []

        def proj_tasks():
            """Deferred output projection of the chunks queued in
            pending_proj, as filler closures. py psum comes from the qk pool
            (its consumers depend only on their own producers, so
            interleaving cannot deadlock the PE FIFO)."""
            if KSTAGE < 5:
                pending_proj.clear()
                return []
            tasks = []
            for m, aoT_ab, aoT_bc in pending_proj:
                def mk(m, aoT_ab, aoT_bc):
                    state = {}

                    def genA():
                        state["y"] = y_pool.tile([128, C], f32, tag="y",
                                                 name="ysb")
                        py = qk_ps.tile([128, 512], f32, tag="qk",
                                        name="py")[:, 0:384]
                        nc.tensor.matmul(py, aoT_ab[:], wpA_sb[:, 0:384],
                                         start=True, stop=False,
                                         skip_group_check=True)
                        nc.tensor.matmul(py, aoT_bc[64:128, :],
                                         wpC_sb[64:128, 0:384],
                                         start=False, stop=True,
                                         skip_group_check=True)
                        nc.vector.tensor_copy(state["y"][:, 0:384], py)

                    def genB():
                        py = qk_ps.tile([128, 512], f32, tag="qk",
                                        name="py")[:, 0:384]
                        nc.tensor.matmul(py, aoT_ab[:], wpA_sb[:, 384:768],
                                         start=True, stop=False,
                                         skip_group_check=True)
                        nc.tensor.matmul(py, aoT_bc[64:128, :],
                                         wpC_sb[64:128, 384:768],
                                         start=False, stop=True,
                                         skip_group_check=True)
                        nc.vector.tensor_copy(state["y"][:, 384:768], py)
                        nc.sync.dma_start(y_d[m * 128:(m + 1) * 128, :],
                                          state["y"][:])

                    return [(320, genA), (320, genB)]
                tasks.extend(mk(m, aoT_ab, aoT_bc))
            pending_proj.clear()
            return tasks

        def emit_attention(j, filler):
            """scores -> exp -> mask -> att@v -> normalize -> transpose for
            q-tile j (heads 0,1 via AB tiles; head 2 via C). att@v for a
            round is emitted one round late so its exp/mask dependencies are
            already satisfied when the PE reaches it. One filler closure
            (qkv of j+1 / deferred proj of j-1) is emitted before each
            round's scores so the PE has independent work while a score
            matmul waits on its psum slot."""
            nk = 4 * j + 4
            # force any work that must precede this tile (its own qkv gens)
            while filler and filler[0][0] <= j:
                filler.popleft()[2]()
            debt = [0.0]

            def pop_filler(deficit):
                debt[0] += deficit
                while filler and debt[0] >= filler[0][1]:
                    tile_req, cost, fn = filler.popleft()
                    fn()
                    debt[0] -= cost

            # att accumulators: 12 of [128, 65] packed into two banks.
            # Interleaved matmul groups share each bank, so no matmul may use
            # start=True (it marks the whole 2KB zero-region pending and
            # corrupts the other groups); the banks are zeroed by DVE memset
            # instead and every attv accumulates.
            accA = at_ps.tile([128, 512], f32, tag="acc", name="accA")
            accB = at_ps.tile([128, 512], f32, tag="acc", name="accB")
            nc.vector.memset(accA[:, 0:455], 0.0)
            nc.vector.memset(accB[:, 0:325], 0.0)

            def acc_ap(h, qc):
                i = h * 4 + qc
                if i < 7:
                    return accA[:, i * 65:(i + 1) * 65]
                return accB[:, (i - 7) * 65:(i - 6) * 65]

            def attv(eb, bank, ki, h):
                if KSTAGE < 3:
                    return
                r = ki - 4 * j
                for qc in range(max(r, 0), 4):
                    nc.tensor.matmul(
                        acc_ap(h, qc),
                        eb[:, bank * 512 + qc * 128:bank * 512 + qc * 128 + 128],
                        v_aug[:, ki * 195 + 65 * h:ki * 195 + 65 * h + 65],
                        start=False, stop=(ki == 4 * j + qc),
                        skip_group_check=True)

            def exp_mask(pr, eb, bank, ki, single=True):
                """exp (trimmed to the valid q-range) + causal mask for one
                512-col bank; with single=False the caller batches the exp."""
                r = ki - 4 * j
                t0 = 128 * max(r, 0)
                if single:
                    nc.scalar.activation(
                        eb[:, bank * 512 + t0:(bank + 1) * 512],
                        pr[:, bank * 512 + t0:(bank + 1) * 512],
                        AF.Exp, scale=0.125)
                if r >= 0:
                    # only the exact-diagonal 128-col block needs masking;
                    # everything past it is strictly below the diagonal
                    sl = eb[:, bank * 512 + t0:bank * 512 + t0 + 128]
                    nc.gpsimd.affine_select(
                        sl, sl, pattern=[[1, 128]],
                        compare_op=mybir.AluOpType.is_ge, fill=0.0,
                        base=0, channel_multiplier=-1)

            # --- heads 0,1: one ki per round, 2 psum banks ---
            prev = None  # (eb, ki) of previous round
            for ki in range(nk):
                r = ki - 4 * j  # >=0 on the diagonal band
                t0 = 128 * max(r, 0)  # valid q-range start within the tile
                act = 2 * (512 - t0) * 0.833 + (370 if t0 else 185)
                pe = 2 * (512 - t0) * 0.417 + 240
                pop_filler((act - pe) * 0.55)
                ksl = bass.ts(ki, 128)
                pr = sc_ps.tile([128, 1024], f32, tag="sc", name="sc")
                for hh in (0, 1):
                    r0 = 64 * hh
                    nc.tensor.matmul(
                        pr[:, hh * 512 + t0:(hh + 1) * 512],
                        kT_AB[r0:r0 + 64, ksl],
                        qT_AB[r0:r0 + 64, j * 512 + t0:(j + 1) * 512],
                        start=True, stop=True)
                eb = eb_pool.tile([128, 1024], bf16, tag="eb", name="eb")
                if t0 == 0:
                    nc.scalar.activation(eb[:], pr[:], AF.Exp, scale=0.125)
                else:
                    src3 = pr[:].rearrange("p (b q) -> p b q",
                                           q=512)[:, :, t0:512]
                    dst3 = eb[:].rearrange("p (b q) -> p b q",
                                           q=512)[:, :, t0:512]
                    nc.scalar.activation(dst3, src3, AF.Exp, scale=0.125)
                exp_mask(pr, eb, 0, ki, single=False)
                exp_mask(pr, eb, 1, ki, single=False)
                if dbg and j == 0 and ki == int(__import__("os").environ.get("KEB", "0")):
                    nc.sync.dma_start(dbg_out["d_eb"], eb[:])
                if prev is not None:
                    peb, pki = prev
                    attv(peb, 0, pki, 0)
                    attv(peb, 1, pki, 1)
                prev = (eb, ki)
            peb, pki = prev
            attv(peb, 0, pki, 0)
            attv(peb, 1, pki, 1)

            # --- head 2: two ki per round using the duplicated C strips ---
            prev = None
            while filler and filler[0][0] <= j + 0.6:
                filler.popleft()[2]()
            for g0 in range(0, nk, 2):
                pop_filler(100)
                pr = sc_ps.tile([128, 1024], f32, tag="sc", name="sc")
                eb = eb_pool.tile([128, 1024], bf16, tag="eb", name="eb")
                diag = g0 + 1 >= 4 * j
                for idx, ki in enumerate((g0, g0 + 1)):
                    r = ki - 4 * j
                    t0 = 128 * max(r, 0)
                    ksl = bass.ts(ki, 128)
                    r0 = 64 * (idx % 2)
                    nc.tensor.matmul(
                        pr[:, idx * 512 + t0:(idx + 1) * 512],
                        kT_C[r0:r0 + 64, ksl],
                        qT_C[r0:r0 + 64, j * 512 + t0:(j + 1) * 512],
                        start=True, stop=True)
                if diag:
                    for idx, ki in enumerate((g0, g0 + 1)):
                        exp_mask(pr, eb, idx, ki)
                else:
                    nc.scalar.activation(eb[:], pr[:], AF.Exp, scale=0.125)
                if prev is not None:
                    peb, pg0 = prev
                    attv(peb, 0, pg0, 2)
                    attv(peb, 1, pg0 + 1, 2)
                prev = (eb, g0)
            peb, pg0 = prev
            attv(peb, 0, pg0, 2)
            attv(peb, 1, pg0 + 1, 2)

            # --- normalize + transpose per 128-chunk; projection deferred.
            # Raw psum->sbuf copies release the acc banks quickly so
            # attv(j+1) is not gated on the full normalization chain. ---
            accs = nrm.tile([128, 780], f32, tag="accs", name="accs")
            nc.vector.tensor_copy(accs[:, 0:455], accA[:, 0:455])
            nc.vector.tensor_copy(accs[:, 455:780], accB[:, 0:325])

            def acc_sb(h, qc):
                i = h * 4 + qc
                return accs[:, i * 65:(i + 1) * 65]

            if dbg and j == 0:
                nc.sync.dma_start(dbg_out["d_accs"], accs[:])

            for qc in range(4 if KSTAGE >= 4 else 0):
                pop_filler(400)
                m = 4 * j + qc
                ao = ao_pool.tile([128, HPC * D], bf16, tag="ao", name="ao")
                for h in range(HPC):
                    a = acc_sb(h, qc)
                    rcp = nrm.tile([128, 1], f32, tag="rcp", name="rcp")
                    nc.vector.reciprocal_approx_fast(out=rcp[:],
                                                     in_=a[:, 64:65])
                    nc.vector.tensor_scalar_mul(
                        ao[:, h * 64:(h + 1) * 64], a[:, 0:64], rcp[:])
                aoT_ab = aoT_pool.tile([128, 128], bf16, tag="tab", name="tab")
                aoT_bc = aoT_pool.tile([128, 128], bf16, tag="tbc", name="tbc")
                nc.sync.dma_start_transpose(aoT_ab[:], ao[:, 0:128])
                nc.sync.dma_start_transpose(aoT_bc[:], ao[:, 64:192])
                if dbg and j == 0 and qc == 0:
                    nc.sync.dma_start(dbg_out["d_ao"], ao[:])
                    nc.sync.dma_start(dbg_out["d_aotab"], aoT_ab[:])
                    nc.sync.dma_start(dbg_out["d_aotbc"], aoT_bc[:])
                pending_proj.append((m, aoT_ab, aoT_bc))
                if j == NT - 1:
                    # last tile: flush inline so the projection overlaps the
                    # remaining normalizations instead of trailing them
                    for _, t in proj_tasks():
                        t()

        from collections import deque
        filler = deque()  # entries: (must_before_tile, cost_ns, closure)
        for _, _, gen in qkv_tasks(0):
            gen()
        # tile 1's q/k generations run up front too: tile 0's attention is
        # too small to host them as filler without starving the ACT engine
        t1 = qkv_tasks(1)
        for _, _, gen in t1[:3]:
            gen()
        for j in range(NT):
            if j == 0:
                filler.extend(t1[3:])
            elif j + 1 < NT:
                filler.extend(qkv_tasks(j + 1))
            filler.extend((j + 2, c, fn) for c, fn in proj_tasks())
            if KSTAGE >= 2:
                emit_attention(j, filler)
            else:
                while filler:
                    filler.popleft()[2]()
        while filler:
            filler.popleft()[2]()
        for _, t in proj_tasks():
            t()
        if dbg:
            nc.sync.dma_start(dbg_out["d_qAB"], qT_AB[:])
            nc.sync.dma_start(dbg_out["d_kAB"], kT_AB[:])
            nc.sync.dma_start(dbg_out["d_qC"], qT_C[:])
            nc.sync.dma_start(dbg_out["d_kC"], kT_C[:])
            nc.sync.dma_start(dbg_out["d_vaug"], v_aug[:])

    nc.compile()
    return nc


_NC_CACHE = {}


def _get_nc(T):
    if T not in _NC_CACHE:
        _NC_CACHE[T] = build_nc(T)
    return _NC_CACHE[T]


def make_core_inputs(x, W_attn, b_attn, W_proj):
    """Host-side prep: per-core input dicts (see module docstring)."""
    B, T, _ = x.shape
    xts = [np.ascontiguousarray(x[b].T) for b in range(B)]
    in_maps = []
    for core in range(N_CORES):
        b = core // (N_CORES // B)
        h0 = HPC * (core % (N_CORES // B))
        ccols = slice(h0 * D, (h0 + 2) * D)      # first two heads
        c2 = slice((h0 + 2) * D, (h0 + 3) * D)   # third head
        # reference splits qkv as (k, q, v): k cols 0:C, q cols C:2C, v 2C:3C
        q01 = W_attn[:, C:2 * C][:, ccols]
        k01 = W_attn[:, 0:C][:, ccols]
        q2 = W_attn[:, C:2 * C][:, c2]
        k2 = W_attn[:, 0:C][:, c2]
        wq = np.ascontiguousarray(
            np.concatenate([q01, k01, q2, k2], axis=1))
        wv = np.ascontiguousarray(
            W_attn[:, 2 * C:3 * C][:, h0 * D:(h0 + HPC) * D])
        bq = np.zeros((128, 3), np.float32)
        bq[:, 0] = b_attn[C:2 * C][ccols]
        bq[:, 1] = b_attn[0:C][ccols]
        bq[0:64, 2] = b_attn[C:2 * C][c2]
        bq[64:128, 2] = b_attn[0:C][c2]
        wp = np.ascontiguousarray(W_proj[h0 * D:(h0 + HPC) * D, :])
        in_maps.append({
            "xt": to_bf16(xts[b]),
            "wq": to_bf16(wq),
            "wv": to_bf16(wv),
            "bq": bq,
            "wp": to_bf16(wp),
        })
    return in_maps


def to_bf16(a):
    import ml_dtypes
    return np.ascontiguousarray(a.astype(ml_dtypes.bfloat16))


def kernel(x, W_attn, b_attn, W_proj, b_proj):
    x = np.asarray(x, dtype=np.float32)
    W_attn = np.asarray(W_attn, dtype=np.float32)
    b_attn = np.asarray(b_attn, dtype=np.float32)
    W_proj = np.asarray(W_proj, dtype=np.float32)
    b_proj = np.asarray(b_proj, dtype=np.float32)
    B, T, _ = x.shape

    nc = _get_nc(T)
    in_maps = make_core_inputs(x, W_attn, b_attn, W_proj)
    res = None
    for attempt in range(3):
        try:
            res = run_bass_kernel_spmd(nc, in_maps, list(range(N_CORES)))
            break
        except Exception:
            # transient NRT_EXEC_UNIT_UNRECOVERABLE has been observed once
            # after a prior crashed process; a retry succeeds
            if attempt == 2:
                raise
    global LAST_RUN
    LAST_RUN = res

    # the v bias contributes b_v @ W_proj to every token (softmax rows sum
    # to 1), folded here instead of inside the kernel
    b_eff = b_proj + b_attn[2 * C:3 * C] @ W_proj

    gpb = N_CORES // B
    out = np.empty((B, T, C), np.float32)
    for b in range(B):
        acc = res.results[b * gpb]["y"].astype(np.float32)
        for g in range(1, gpb):
            acc = acc + res.results[b * gpb + g]["y"]
        out[b] = acc + b_eff[None, :]
    return out


# revision 37
# speedup vs baseline: 1.0312x; 1.0312x over previous
"""Causal self-attention (B=2, T=4096, C=768, H=12) on 8 TRN2 NeuronCores.

Sharding: batch x head-group. Core c handles batch b=c//4 and heads
h0..h0+2 where h0 = 3*(c%4). Each core computes the qkv projection for
its 3 heads, full causal attention, and a partial output projection; the
host sums the 4 partials per batch and adds the (v-bias-folded)
projection bias.

All matmul operands are bf16 (1 cycle/row on the PE at any moving size).
q/k live transposed ([D, T]) feeding the scores matmul; v is computed in
natural token-major layout and packed into v_aug [k, 65*3] with a ones
column per head so the softmax denominator falls out of att@v as output
column 64. att@v runs in [q, d] orientation (stationary eb^T chunk
[k,128q], moving v_aug [k,65]) so each 128x128 block costs only 65 PE
cycles and the denominator lands as a per-partition column, normalized
with reciprocal + tensor_scalar (no partition broadcast). The normalized
ao [tok, hd] is flipped to [hd, tok] with DMA-engine xbar transposes and
fed to the output projection as the stationary operand.

Causality: scores/exp are only computed for k-tiles at or below the
diagonal, trimmed to the valid q-range on the diagonal band; the
remaining intra-block mask is applied in-place on eb by gpsimd
affine_select. The v bias is algebraically folded into the host-side
projection bias (softmax weights sum to 1).
"""

import sys

for _p in ("/opt/trn_rl_repo",):
    if _p not in sys.path:
        sys.path.insert(0, _p)

from contextlib import ExitStack

import numpy as np

import concourse.bass as bass  # noqa: F401
import concourse.mybir as mybir
import concourse.tile as tile
from concourse import bacc
from concourse.bass_utils import run_bass_kernel_spmd

f32 = mybir.dt.float32
bf16 = mybir.dt.bfloat16
AF = mybir.ActivationFunctionType

C = 768
D = 64
N_HEAD = 12
HPC = 3  # heads per core
N_CORES = 8

# wq column slots: q01 | k01 | (q2 stacked over k2)
QK_SLOTS = [(0, 128), (128, 256), (256, 384)]


def build_nc(T):
    NT = T // 512  # q tiles
    KT = T // 128  # k tiles / token chunks
    CK = C // 128  # contraction chunks

    nc = bacc.Bacc("TRN2", target_bir_lowering=False, debug=False,
                   num_devices=N_CORES)
    # merged partition-major layouts: [p, c*cols + f] so each load is ONE
    # DMA instead of CK of them (HWDGE is a single serial device)
    xt_d = nc.dram_tensor("xt", [128, CK * T], bf16, kind="ExternalInput").ap()
    wq_d = nc.dram_tensor("wq", [128, CK * 384], bf16,
                          kind="ExternalInput").ap()
    wv_d = nc.dram_tensor("wv", [128, CK * HPC * D], bf16,
                          kind="ExternalInput").ap()
    bq_d = nc.dram_tensor("bq", [128, 3], f32, kind="ExternalInput").ap()
    wp_d = nc.dram_tensor("wp", [HPC * D, C], bf16, kind="ExternalInput").ap()
    y_d = nc.dram_tensor("y", [T, C], f32, kind="ExternalOutput").ap()
    import os
    dbg = os.environ.get("KDBG") == "1"
    dbg_out = {}
    if dbg:
        for nm, shp, dt in [("d_qAB", [128, T], bf16), ("d_kAB", [128, T], bf16),
                            ("d_qC", [128, T], bf16), ("d_kC", [128, T], bf16),
                            ("d_vaug", [128, 32 * 195], bf16),
                            ("d_eb", [128, 1024], bf16),
                            ("d_accs", [128, 780], f32),
                            ("d_ao", [128, 192], bf16),
                            ("d_aotab", [128, 128], bf16),
                            ("d_aotbc", [128, 128], bf16)]:
            dbg_out[nm] = nc.dram_tensor(nm, shp, dt, kind="ExternalOutput").ap()

    with tile.TileContext(nc) as tc, ExitStack() as ctx:
        sb = ctx.enter_context(tc.tile_pool(name="sb", bufs=1))

        # persistent tensors
        bq_sb = sb.tile([128, 3], f32, tag="bq")
        qT_AB = sb.tile([128, T], bf16, tag="qAB")
        kT_AB = sb.tile([128, T], bf16, tag="kAB")
        qT_C = sb.tile([128, T], bf16, tag="qC")
        kT_C = sb.tile([128, T], bf16, tag="kC")
        v_aug = sb.tile([128, KT * 195], bf16, tag="vaug")
        wq_sb2 = sb.tile([128, CK * 384], bf16, tag="wq")
        wv_sb2 = sb.tile([128, CK * HPC * D], bf16, tag="wv")
        wq_sb = [wq_sb2[:, c * 384:(c + 1) * 384] for c in range(CK)]
        wv_sb = [wv_sb2[:, c * HPC * D:(c + 1) * HPC * D] for c in range(CK)]
        wpA_sb = sb.tile([128, C], bf16, tag="wpA")   # heads 0,1
        # head 2 lives at partitions 64:128 to match aoT_bc's h2 strip
        wpC_sb = sb.tile([128, C], bf16, tag="wpC")

        nc.sync.dma_start(bq_sb[:], bq_d)
        # ones columns of v_aug (cols 64,129,194 mod 195) come from this
        # blanket fill; the v copies below overwrite the 64-col data slices.
        nc.gpsimd.memset(v_aug[:], 1.0)

        # PE warm-up: ~3.5us of dummy matmuls bridge the initial xt DMA
        # latency so the p-state ramp completes before the first real matmul
        warm = sb.tile([128, 512], bf16, tag="warm")
        nc.vector.memset(warm[:], 0.5)
        with tc.tile_pool(name="wups", bufs=1, space="PSUM") as wu_ps:
            wu = wu_ps.tile([128, 512], f32, tag="wu")
            for _ in range(12):
                nc.tensor.matmul(wu[:], warm[:, 0:128], warm[:],
                                 start=True, stop=True,
                                 skip_group_check=True)

        # xt strips: whole [128, T] rows per contraction chunk; the j=0
        # slice loads first so the pipeline can start early.
        xt_sb2 = sb.tile([128, CK * T], bf16, tag="xts")
        xt_sb = [xt_sb2[:, c * T:(c + 1) * T] for c in range(CK)]
        xt3 = xt_sb2[:].rearrange("p (c t) -> p c t", t=T)
        xt3_d = xt_d.rearrange("p (c t) -> p c t", t=T)

        def xt_wave(a, b):
            nc.sync.dma_start(xt3[:, :, a:b], xt3_d[:, :, a:b])

        nc.sync.dma_start(wq_sb2[:], wq_d)
        xt_wave(0, 512)
        xt_wave(512, 1024)
        nc.sync.dma_start(wv_sb2[:], wv_d)
        nc.sync.dma_start(wpA_sb[:], wp_d[0:128, :])
        nc.sync.dma_start(wpC_sb[64:128, :], wp_d[128:192, :])
        for half in range(1, 4):
            xt_wave(half * 1024, (half + 1) * 1024)
        # PSUM budget (8 banks): sc 2x2 + acc 2 + qkv 2. The acc slots
        # also serve the deferred projection (they are idle between norm(j)
        # and attv(j+1)); qkv rotates its 5 generations through 2 banks.
        sc_ps = ctx.enter_context(
            tc.tile_pool(name="scps", bufs=2, space="PSUM"))
        at_ps = ctx.enter_context(
            tc.tile_pool(name="atps", bufs=2, space="PSUM"))
        qk_ps = ctx.enter_context(
            tc.tile_pool(name="qkps", bufs=2, space="PSUM"))
        eb_pool = ctx.enter_context(tc.tile_pool(name="ebp", bufs=12))
        ao_pool = ctx.enter_context(tc.tile_pool(name="aop", bufs=6))
        aoT_pool = ctx.enter_context(tc.tile_pool(name="aotp", bufs=6))
        y_pool = ctx.enter_context(tc.tile_pool(name="yp", bufs=4))
        nrm = ctx.enter_context(tc.tile_pool(name="nrm", bufs=4))

        def qkv_tasks(j):
            """qkv projection for q-tile j as a list of filler closures (one
            per psum generation) so the PE work can interleave between
            attention rounds of the previous tile."""
            jsl = bass.ts(j, 512)
            xt_t = [xt_sb[c][:, jsl] for c in range(CK)]

            def qk_slot(ps_reg, s):
                c0, c1 = QK_SLOTS[s]
                for c in range(CK):
                    nc.tensor.matmul(ps_reg, wq_sb[c][:, c0:c1], xt_t[c],
                                     start=(c == 0), stop=(c == CK - 1),
                                     skip_group_check=True)

            def v_chunk(ps_reg, tc_):
                for c in range(CK):
                    nc.tensor.matmul(ps_reg,
                                     xt_t[c][:, tc_ * 128:(tc_ + 1) * 128],
                                     wv_sb[c],
                                     start=(c == 0), stop=(c == CK - 1),
                                     skip_group_check=True)

            def v_store(ps_reg, tc_):
                base = (4 * j + tc_) * 195
                dst = v_aug[:, base:base + 195].rearrange(
                    "p (h c) -> p h c", c=65)[:, :, 0:64]
                nc.vector.tensor_copy(
                    dst, ps_reg.rearrange("p (h c) -> p h c", c=64))

            def gen1():
                g = qk_ps.tile([128, 512], f32, tag="qk", name="qkv1")
                qk_slot(g[:], 0)
                nc.vector.tensor_scalar_add(qT_AB[:, jsl], g[:],
                                            bq_sb[:, 0:1])

            def gen2():
                g = qk_ps.tile([128, 512], f32, tag="qk", name="qkv2")
                qk_slot(g[:], 1)
                nc.vector.tensor_scalar_add(kT_AB[:, jsl], g[:],
                                            bq_sb[:, 1:2])

            def gen3():
                g = qk_ps.tile([128, 512], f32, tag="qk", name="qkv3")
                qk_slot(g[:], 2)
                # q2 rows 0:64, k2 rows 64:128 stacked in one bank
                nc.vector.tensor_scalar_add(qT_C[0:64, jsl], g[0:64, :],
                                            bq_sb[0:64, 2:3])
                nc.vector.tensor_scalar_add(kT_C[64:128, jsl], g[64:128, :],
                                            bq_sb[64:128, 2:3])
                # duplicate head-2 q/k into the other 64-partition strip so
                # the scores matmul can alternate strips (operands must share
                # the partition range)
                nc.sync.dma_start(qT_C[64:128, jsl], qT_C[0:64, jsl])
                nc.sync.dma_start(kT_C[0:64, jsl], kT_C[64:128, jsl])

            def mkv(tc_):
                # one accumulation group per psum generation: a start=True
                # in a bank wipes other in-flight groups' pending state
                def gen():
                    g = qk_ps.tile([128, 512], f32, tag="qk",
                                   name=f"qkv4_{tc_}")
                    v_chunk(g[:, 0:192], tc_)
                    v_store(g[:, 0:192], tc_)
                return gen

            # (must_before_tile, cost, closure): gen1/2/3 must run early
            # enough that tile j's scores never wait on them; the v chunks
            # by the start of attention(j)
            gens = [gen1, gen2, gen3] + [mkv(t) for t in range(4)]
            tags = [j - 0.4] * 3 + [j] * 4
            costs = [1280] * 3 + [480] * 4
            return list(zip(tags, costs, gens))

        import os
        KSTAGE = int(os.environ.get("KSTAGE", "5"))

        # chunks whose normalize+transpose ran in iteration j; their output
        # projection is deferred into iteration j+1 so the PE never waits on
        # a transpose DMA at the head of its in-order queue.
        pending_proj = []

        def proj_tasks():
            """Deferred output projection of the chunks queued in
            pending_proj, as filler closures. py psum comes from the qk pool
            (its consumers depend only on their own producers, so
            interleaving cannot deadlock the PE FIFO)."""
            if KSTAGE < 5:
                pending_proj.clear()
                return []
            tasks = []
            for m, aoT_ab, aoT_bc in pending_proj:
                def mk(m, aoT_ab, aoT_bc):
                    state = {}

                    def genA():
                        state["y"] = y_pool.tile([128, C], f32, tag="y",
                                                 name="ysb")
                        py = qk_ps.tile([128, 512], f32, tag="qk",
                                        name="py")[:, 0:384]
                        nc.tensor.matmul(py, aoT_ab[:], wpA_sb[:, 0:384],
                                         start=True, stop=False,
                                         skip_group_check=True)
                        nc.tensor.matmul(py, aoT_bc[64:128, :],
                                         wpC_sb[64:128, 0:384],
                                         start=False, stop=True,
                                         skip_group_check=True)
                        nc.vector.tensor_copy(state["y"][:, 0:384], py)

                    def genB():
                        py = qk_ps.tile([128, 512], f32, tag="qk",
                                        name="py")[:, 0:384]
                        nc.tensor.matmul(py, aoT_ab[:], wpA_sb[:, 384:768],
                                         start=True, stop=False,
                                         skip_group_check=True)
                        nc.tensor.matmul(py, aoT_bc[64:128, :],
                                         wpC_sb[64:128, 384:768],
                                         start=False, stop=True,
                                         skip_group_check=True)
                        nc.vector.tensor_copy(state["y"][:, 384:768], py)
                        nc.sync.dma_start(y_d[m * 128:(m + 1) * 128, :],
                                          state["y"][:])

                    return [(320, genA), (320, genB)]
                tasks.extend(mk(m, aoT_ab, aoT_bc))
            pending_proj.clear()
            return tasks

        def emit_attention(j, filler):
            """scores -> exp -> mask -> att@v -> normalize -> transpose for
            q-tile j (heads 0,1 via AB tiles; head 2 via C). att@v for a
            round is emitted one round late so its exp/mask dependencies are
            already satisfied when the PE reaches it. One filler closure
            (qkv of j+1 / deferred proj of j-1) is emitted before each
            round's scores so the PE has independent work while a score
            matmul waits on its psum slot."""
            nk = 4 * j + 4
            # force any work that must precede this tile (its own qkv gens)
            while filler and filler[0][0] <= j:
                filler.popleft()[2]()
            debt = [0.0]

            def pop_filler(deficit):
                debt[0] += deficit
                while filler and debt[0] >= filler[0][1]:
                    tile_req, cost, fn = filler.popleft()
                    fn()
                    debt[0] -= cost

            # att accumulators: 12 of [128, 65] packed into two banks.
            # Interleaved matmul groups share each bank, so no matmul may use
            # start=True (it marks the whole 2KB zero-region pending and
            # corrupts the other groups); the banks are zeroed by DVE memset
            # instead and every attv accumulates.
            accA = at_ps.tile([128, 512], f32, tag="acc", name="accA")
            accB = at_ps.tile([128, 512], f32, tag="acc", name="accB")
            nc.vector.memset(accA[:, 0:455], 0.0)
            nc.vector.memset(accB[:, 0:325], 0.0)

            def acc_ap(h, qc):
                i = h * 4 + qc
                if i < 7:
                    return accA[:, i * 65:(i + 1) * 65]
                return accB[:, (i - 7) * 65:(i - 6) * 65]

            def attv(eb, bank, ki, h):
                if KSTAGE < 3:
                    return
                r = ki - 4 * j
                for qc in range(max(r, 0), 4):
                    nc.tensor.matmul(
                        acc_ap(h, qc),
                        eb[:, bank * 512 + qc * 128:bank * 512 + qc * 128 + 128],
                        v_aug[:, ki * 195 + 65 * h:ki * 195 + 65 * h + 65],
                        start=False, stop=(ki == 4 * j + qc),
                        skip_group_check=True)

            def exp_mask(pr, eb, bank, ki, single=True):
                """exp (trimmed to the valid q-range) + causal mask for one
                512-col bank; with single=False the caller batches the exp."""
                r = ki - 4 * j
                t0 = 128 * max(r, 0)
                if single:
                    nc.scalar.activation(
                        eb[:, bank * 512 + t0:(bank + 1) * 512],
                        pr[:, bank * 512 + t0:(bank + 1) * 512],
                        AF.Exp, scale=0.125)
                if r >= 0:
                    # only the exact-diagonal 128-col block needs masking;
                    # everything past it is strictly below the diagonal
                    sl = eb[:, bank * 512 + t0:bank * 512 + t0 + 128]
                    nc.gpsimd.affine_select(
                        sl, sl, pattern=[[1, 128]],
                        compare_op=mybir.AluOpType.is_ge, fill=0.0,
                        base=0, channel_multiplier=-1)

            # --- heads 0,1: one ki per round, 2 psum banks ---
            from collections import deque as _dq
            pend = _dq()  # (eb, ki) of recent rounds; attv runs 2 late
            for ki in range(nk):
                r = ki - 4 * j  # >=0 on the diagonal band
                t0 = 128 * max(r, 0)  # valid q-range start within the tile
                act = 2 * (512 - t0) * 0.833 + (370 if t0 else 185)
                pe = 2 * (512 - t0) * 0.417 + 240
                pop_filler((act - pe) * 0.55)
                ksl = bass.ts(ki, 128)
                pr = sc_ps.tile([128, 1024], f32, tag="sc", name="sc")
                for hh in (0, 1):
                    r0 = 64 * hh
                    nc.tensor.matmul(
                        pr[:, hh * 512 + t0:(hh + 1) * 512],
                        kT_AB[r0:r0 + 64, ksl],
                        qT_AB[r0:r0 + 64, j * 512 + t0:(j + 1) * 512],
                        start=True, stop=True)
                eb = eb_pool.tile([128, 1024], bf16, tag="eb", name="eb")
                if t0 == 0:
                    nc.scalar.activation(eb[:], pr[:], AF.Exp, scale=0.125)
                else:
                    src3 = pr[:].rearrange("p (b q) -> p b q",
                                           q=512)[:, :, t0:512]
                    dst3 = eb[:].rearrange("p (b q) -> p b q",
                                           q=512)[:, :, t0:512]
                    nc.scalar.activation(dst3, src3, AF.Exp, scale=0.125)
                exp_mask(pr, eb, 0, ki, single=False)
                exp_mask(pr, eb, 1, ki, single=False)
                if dbg and j == 0 and ki == int(__import__("os").environ.get("KEB", "0")):
                    nc.sync.dma_start(dbg_out["d_eb"], eb[:])
                pend.append((eb, ki))
                if len(pend) > 2:
                    peb, pki = pend.popleft()
                    attv(peb, 0, pki, 0)
                    attv(peb, 1, pki, 1)
            while pend:
                peb, pki = pend.popleft()
                attv(peb, 0, pki, 0)
                attv(peb, 1, pki, 1)

            # --- head 2: two ki per round using the duplicated C strips ---
            pend = _dq()
            while filler and filler[0][0] <= j + 0.6:
                filler.popleft()[2]()
            for g0 in range(0, nk, 2):
                pop_filler(100)
                pr = sc_ps.tile([128, 1024], f32, tag="sc", name="sc")
                eb = eb_pool.tile([128, 1024], bf16, tag="eb", name="eb")
                diag = g0 + 1 >= 4 * j
                for idx, ki in enumerate((g0, g0 + 1)):
                    r = ki - 4 * j
                    t0 = 128 * max(r, 0)
                    ksl = bass.ts(ki, 128)
                    r0 = 64 * (idx % 2)
                    nc.tensor.matmul(
                        pr[:, idx * 512 + t0:(idx + 1) * 512],
                        kT_C[r0:r0 + 64, ksl],
                        qT_C[r0:r0 + 64, j * 512 + t0:(j + 1) * 512],
                        start=True, stop=True)
                if diag:
                    for idx, ki in enumerate((g0, g0 + 1)):
                        exp_mask(pr, eb, idx, ki)
                else:
                    nc.scalar.activation(eb[:], pr[:], AF.Exp, scale=0.125)
                pend.append((eb, g0))
                if len(pend) > 2:
                    peb, pg0 = pend.popleft()
                    attv(peb, 0, pg0, 2)
                    attv(peb, 1, pg0 + 1, 2)
            while pend:
                peb, pg0 = pend.popleft()
                attv(peb, 0, pg0, 2)
                attv(peb, 1, pg0 + 1, 2)

            # --- normalize + transpose per 128-chunk; projection deferred.
            # Raw psum->sbuf copies release the acc banks quickly so
            # attv(j+1) is not gated on the full normalization chain. ---
            accs = nrm.tile([128, 780], f32, tag="accs", name="accs")
            nc.vector.tensor_copy(accs[:, 0:455], accA[:, 0:455])
            nc.vector.tensor_copy(accs[:, 455:780], accB[:, 0:325])

            def acc_sb(h, qc):
                i = h * 4 + qc
                return accs[:, i * 65:(i + 1) * 65]

            if dbg and j == 0:
                nc.sync.dma_start(dbg_out["d_accs"], accs[:])

            for qc in range(4 if KSTAGE >= 4 else 0):
                pop_filler(400)
                m = 4 * j + qc
                ao = ao_pool.tile([128, HPC * D], bf16, tag="ao", name="ao")
                for h in range(HPC):
                    a = acc_sb(h, qc)
                    rcp = nrm.tile([128, 1], f32, tag="rcp", name="rcp")
                    nc.vector.reciprocal_approx_fast(out=rcp[:],
                                                     in_=a[:, 64:65])
                    nc.vector.tensor_scalar_mul(
                        ao[:, h * 64:(h + 1) * 64], a[:, 0:64], rcp[:])
                aoT_ab = aoT_pool.tile([128, 128], bf16, tag="tab", name="tab")
                aoT_bc = aoT_pool.tile([128, 128], bf16, tag="tbc", name="tbc")
                nc.sync.dma_start_transpose(aoT_ab[:], ao[:, 0:128])
                nc.sync.dma_start_transpose(aoT_bc[:], ao[:, 64:192])
                if dbg and j == 0 and qc == 0:
                    nc.sync.dma_start(dbg_out["d_ao"], ao[:])
                    nc.sync.dma_start(dbg_out["d_aotab"], aoT_ab[:])
                    nc.sync.dma_start(dbg_out["d_aotbc"], aoT_bc[:])
                pending_proj.append((m, aoT_ab, aoT_bc))
                if j == NT - 1:
                    # last tile: flush inline so the projection overlaps the
                    # remaining normalizations instead of trailing them
                    for _, t in proj_tasks():
                        t()

        from collections import deque
        filler = deque()  # entries: (must_before_tile, cost_ns, closure)
        for _, _, gen in qkv_tasks(0):
            gen()
        # tile 1's q/k generations run up front too: tile 0's attention is
        # too small to host them as filler without starving the ACT engine
        t1 = qkv_tasks(1)
        for _, _, gen in t1[:3]:
            gen()
        for j in range(NT):
            if j == 0:
                filler.extend(t1[3:])
            elif j + 1 < NT:
                filler.extend(qkv_tasks(j + 1))
            filler.extend((j + 2, c, fn) for c, fn in proj_tasks())
            if KSTAGE >= 2:
                emit_attention(j, filler)
            else:
                while filler:
                    filler.popleft()[2]()
        while filler:
            filler.popleft()[2]()
        for _, t in proj_tasks():
            t()
        if dbg:
            nc.sync.dma_start(dbg_out["d_qAB"], qT_AB[:])
            nc.sync.dma_start(dbg_out["d_kAB"], kT_AB[:])
            nc.sync.dma_start(dbg_out["d_qC"], qT_C[:])
            nc.sync.dma_start(dbg_out["d_kC"], kT_C[:])
            nc.sync.dma_start(dbg_out["d_vaug"], v_aug[:])

    nc.compile()
    return nc


_NC_CACHE = {}


def _get_nc(T):
    if T not in _NC_CACHE:
        _NC_CACHE[T] = build_nc(T)
    return _NC_CACHE[T]


def make_core_inputs(x, W_attn, b_attn, W_proj):
    """Host-side prep: per-core input dicts (see module docstring)."""
    B, T, _ = x.shape
    xts = [np.ascontiguousarray(x[b].T) for b in range(B)]
    in_maps = []
    for core in range(N_CORES):
        b = core // (N_CORES // B)
        h0 = HPC * (core % (N_CORES // B))
        ccols = slice(h0 * D, (h0 + 2) * D)      # first two heads
        c2 = slice((h0 + 2) * D, (h0 + 3) * D)   # third head
        # reference splits qkv as (k, q, v): k cols 0:C, q cols C:2C, v 2C:3C
        q01 = W_attn[:, C:2 * C][:, ccols]
        k01 = W_attn[:, 0:C][:, ccols]
        q2 = W_attn[:, C:2 * C][:, c2]
        k2 = W_attn[:, 0:C][:, c2]
        wq = np.ascontiguousarray(
            np.concatenate([q01, k01, q2, k2], axis=1))
        wv = np.ascontiguousarray(
            W_attn[:, 2 * C:3 * C][:, h0 * D:(h0 + HPC) * D])
        bq = np.zeros((128, 3), np.float32)
        bq[:, 0] = b_attn[C:2 * C][ccols]
        bq[:, 1] = b_attn[0:C][ccols]
        bq[0:64, 2] = b_attn[C:2 * C][c2]
        bq[64:128, 2] = b_attn[0:C][c2]
        wp = np.ascontiguousarray(W_proj[h0 * D:(h0 + HPC) * D, :])
        def pmaj(a):
            # [CK*128, f] -> [128, CK*f]
            f = a.shape[1]
            return np.ascontiguousarray(
                a.reshape(-1, 128, f).transpose(1, 0, 2).reshape(128, -1))

        in_maps.append({
            "xt": to_bf16(pmaj(xts[b])),
            "wq": to_bf16(pmaj(wq)),
            "wv": to_bf16(pmaj(wv)),
            "bq": bq,
            "wp": to_bf16(wp),
        })
    return in_maps


def to_bf16(a):
    import ml_dtypes
    return np.ascontiguousarray(a.astype(ml_dtypes.bfloat16))


def kernel(x, W_attn, b_attn, W_proj, b_proj):
    x = np.asarray(x, dtype=np.float32)
    W_attn = np.asarray(W_attn, dtype=np.float32)
    b_attn = np.asarray(b_attn, dtype=np.float32)
    W_proj = np.asarray(W_proj, dtype=np.float32)
    b_proj = np.asarray(b_proj, dtype=np.float32)
    B, T, _ = x.shape

    nc = _get_nc(T)
    in_maps = make_core_inputs(x, W_attn, b_attn, W_proj)
    res = None
    for attempt in range(3):
        try:
            res = run_bass_kernel_spmd(nc, in_maps, list(range(N_CORES)))
            break
        except Exception:
            # transient NRT_EXEC_UNIT_UNRECOVERABLE has been observed once
            # after a prior crashed process; a retry succeeds
            if attempt == 2:
                raise
    global LAST_RUN
    LAST_RUN = res

    # the v bias contributes b_v @ W_proj to every token (softmax rows sum
    # to 1), folded here instead of inside the kernel
    b_eff = b_proj + b_attn[2 * C:3 * C] @ W_proj

    gpb = N_CORES // B
    out = np.empty((B, T, C), np.float32)
    for b in range(B):
        acc = res.results[b * gpb]["y"].astype(np.float32)
        for g in range(1, gpb):
            acc = acc + res.results[b * gpb + g]["y"]
        out[b] = acc + b_eff[None, :]
    return out


# revision 42
# speedup vs baseline: 1.0393x; 1.0079x over previous
"""Causal self-attention (B=2, T=4096, C=768, H=12) on 8 TRN2 NeuronCores.

Sharding: batch x head-group. Core c handles batch b=c//4 and heads
h0..h0+2 where h0 = 3*(c%4). Each core computes the qkv projection for
its 3 heads, full causal attention, and a partial output projection; the
host sums the 4 partials per batch and adds the (v-bias-folded)
projection bias.

All matmul operands are bf16 (1 cycle/row on the PE at any moving size).
q/k live transposed ([D, T]) feeding the scores matmul; v is computed in
natural token-major layout and packed into v_aug [k, 65*3] with a ones
column per head so the softmax denominator falls out of att@v as output
column 64. att@v runs in [q, d] orientation (stationary eb^T chunk
[k,128q], moving v_aug [k,65]) so each 128x128 block costs only 65 PE
cycles and the denominator lands as a per-partition column, normalized
with reciprocal + tensor_scalar (no partition broadcast). The normalized
ao [tok, hd] is flipped to [hd, tok] with DMA-engine xbar transposes and
fed to the output projection as the stationary operand.

Causality: scores/exp are only computed for k-tiles at or below the
diagonal, trimmed to the valid q-range on the diagonal band; the
remaining intra-block mask is applied in-place on eb by gpsimd
affine_select. The v bias is algebraically folded into the host-side
projection bias (softmax weights sum to 1).
"""

import sys

for _p in ("/opt/trn_rl_repo",):
    if _p not in sys.path:
        sys.path.insert(0, _p)

from contextlib import ExitStack

import numpy as np

import concourse.bass as bass  # noqa: F401
import concourse.mybir as mybir
import concourse.tile as tile
from concourse import bacc
from concourse.bass_utils import run_bass_kernel_spmd

f32 = mybir.dt.float32
bf16 = mybir.dt.bfloat16
AF = mybir.ActivationFunctionType

C = 768
D = 64
N_HEAD = 12
HPC = 3  # heads per core
N_CORES = 8

# wq column slots: q01 | k01 | (q2 stacked over k2)
QK_SLOTS = [(0, 128), (128, 256), (256, 384)]


def build_nc(T):
    NT = T // 512  # q tiles
    KT = T // 128  # k tiles / token chunks
    CK = C // 128  # contraction chunks

    nc = bacc.Bacc("TRN2", target_bir_lowering=False, debug=False,
                   num_devices=N_CORES)
    # merged partition-major layouts: [p, c*cols + f] so each load is ONE
    # DMA instead of CK of them (HWDGE is a single serial device)
    xt_d = nc.dram_tensor("xt", [128, CK * T], bf16, kind="ExternalInput").ap()
    wq_d = nc.dram_tensor("wq", [128, CK * 384], bf16,
                          kind="ExternalInput").ap()
    wv_d = nc.dram_tensor("wv", [128, CK * HPC * D], bf16,
                          kind="ExternalInput").ap()
    bq_d = nc.dram_tensor("bq", [128, 3], f32, kind="ExternalInput").ap()
    wp_d = nc.dram_tensor("wp", [HPC * D, C], bf16, kind="ExternalInput").ap()
    y_d = nc.dram_tensor("y", [T, C], bf16, kind="ExternalOutput").ap()
    import os
    dbg = os.environ.get("KDBG") == "1"
    dbg_out = {}
    if dbg:
        for nm, shp, dt in [("d_qAB", [128, T], bf16), ("d_kAB", [128, T], bf16),
                            ("d_qC", [128, T], bf16), ("d_kC", [128, T], bf16),
                            ("d_vaug", [128, 32 * 195], bf16),
                            ("d_eb", [128, 1024], bf16),
                            ("d_accs", [128, 780], f32),
                            ("d_ao", [128, 192], bf16),
                            ("d_aotab", [128, 128], bf16),
                            ("d_aotbc", [128, 128], bf16)]:
            dbg_out[nm] = nc.dram_tensor(nm, shp, dt, kind="ExternalOutput").ap()

    with tile.TileContext(nc) as tc, ExitStack() as ctx:
        sb = ctx.enter_context(tc.tile_pool(name="sb", bufs=1))

        # persistent tensors
        bq_sb = sb.tile([128, 3], f32, tag="bq")
        qT_AB = sb.tile([128, T], bf16, tag="qAB")
        kT_AB = sb.tile([128, T], bf16, tag="kAB")
        qT_C = sb.tile([128, T], bf16, tag="qC")
        kT_C = sb.tile([128, T], bf16, tag="kC")
        v_aug = sb.tile([128, KT * 195], bf16, tag="vaug")
        wq_sb2 = sb.tile([128, CK * 384], bf16, tag="wq")
        wv_sb2 = sb.tile([128, CK * HPC * D], bf16, tag="wv")
        wq_sb = [wq_sb2[:, c * 384:(c + 1) * 384] for c in range(CK)]
        wv_sb = [wv_sb2[:, c * HPC * D:(c + 1) * HPC * D] for c in range(CK)]
        wpA_sb = sb.tile([128, C], bf16, tag="wpA")   # heads 0,1
        # head 2 lives at partitions 64:128 to match aoT_bc's h2 strip
        wpC_sb = sb.tile([128, C], bf16, tag="wpC")

        nc.sync.dma_start(bq_sb[:], bq_d)
        # ones columns of v_aug (cols 64,129,194 mod 195) come from this
        # blanket fill; the v copies below overwrite the 64-col data slices.
        nc.gpsimd.memset(v_aug[:], 1.0)

        # PE warm-up: ~3.5us of dummy matmuls bridge the initial xt DMA
        # latency so the p-state ramp completes before the first real matmul
        warm = sb.tile([128, 512], bf16, tag="warm")
        nc.vector.memset(warm[:], 0.5)
        with tc.tile_pool(name="wups", bufs=1, space="PSUM") as wu_ps:
            wu = wu_ps.tile([128, 512], f32, tag="wu")
            for _ in range(12):
                nc.tensor.matmul(wu[:], warm[:, 0:128], warm[:],
                                 start=True, stop=True,
                                 skip_group_check=True)

        # xt strips: whole [128, T] rows per contraction chunk; the j=0
        # slice loads first so the pipeline can start early.
        xt_sb2 = sb.tile([128, CK * T], bf16, tag="xts")
        xt_sb = [xt_sb2[:, c * T:(c + 1) * T] for c in range(CK)]
        xt3 = xt_sb2[:].rearrange("p (c t) -> p c t", t=T)
        xt3_d = xt_d.rearrange("p (c t) -> p c t", t=T)

        def xt_wave(a, b):
            nc.sync.dma_start(xt3[:, :, a:b], xt3_d[:, :, a:b])

        nc.sync.dma_start(wq_sb2[:], wq_d)
        xt_wave(0, 512)
        xt_wave(512, 1024)
        nc.sync.dma_start(wv_sb2[:], wv_d)
        nc.sync.dma_start(wpA_sb[:], wp_d[0:128, :])
        nc.sync.dma_start(wpC_sb[64:128, :], wp_d[128:192, :])
        for half in range(1, 4):
            xt_wave(half * 1024, (half + 1) * 1024)
        # PSUM budget (8 banks): sc 2x2 + acc 2 + qkv 2. The acc slots
        # also serve the deferred projection (they are idle between norm(j)
        # and attv(j+1)); qkv rotates its 5 generations through 2 banks.
        sc_ps = ctx.enter_context(
            tc.tile_pool(name="scps", bufs=2, space="PSUM"))
        at_ps = ctx.enter_context(
            tc.tile_pool(name="atps", bufs=2, space="PSUM"))
        qk_ps = ctx.enter_context(
            tc.tile_pool(name="qkps", bufs=2, space="PSUM"))
        eb_pool = ctx.enter_context(tc.tile_pool(name="ebp", bufs=12))
        ao_pool = ctx.enter_context(tc.tile_pool(name="aop", bufs=6))
        aoT_pool = ctx.enter_context(tc.tile_pool(name="aotp", bufs=6))
        y_pool = ctx.enter_context(tc.tile_pool(name="yp", bufs=4))
        nrm = ctx.enter_context(tc.tile_pool(name="nrm", bufs=4))

        def qkv_tasks(j):
            """qkv projection for q-tile j as a list of filler closures (one
            per psum generation) so the PE work can interleave between
            attention rounds of the previous tile."""
            jsl = bass.ts(j, 512)
            xt_t = [xt_sb[c][:, jsl] for c in range(CK)]

            def qk_slot(ps_reg, s):
                c0, c1 = QK_SLOTS[s]
                for c in range(CK):
                    nc.tensor.matmul(ps_reg, wq_sb[c][:, c0:c1], xt_t[c],
                                     start=(c == 0), stop=(c == CK - 1),
                                     skip_group_check=True)

            def v_chunk(ps_reg, tc_):
                for c in range(CK):
                    nc.tensor.matmul(ps_reg,
                                     xt_t[c][:, tc_ * 128:(tc_ + 1) * 128],
                                     wv_sb[c],
                                     start=(c == 0), stop=(c == CK - 1),
                                     skip_group_check=True)

            def v_store(ps_reg, tc_):
                base = (4 * j + tc_) * 195
                dst = v_aug[:, base:base + 195].rearrange(
                    "p (h c) -> p h c", c=65)[:, :, 0:64]
                nc.vector.tensor_copy(
                    dst, ps_reg.rearrange("p (h c) -> p h c", c=64))

            def gen1():
                g = qk_ps.tile([128, 512], f32, tag="qk", name="qkv1")
                qk_slot(g[:], 0)
                nc.vector.tensor_scalar_add(qT_AB[:, jsl], g[:],
                                            bq_sb[:, 0:1])

            def gen2():
                g = qk_ps.tile([128, 512], f32, tag="qk", name="qkv2")
                qk_slot(g[:], 1)
                nc.vector.tensor_scalar_add(kT_AB[:, jsl], g[:],
                                            bq_sb[:, 1:2])

            def gen3():
                g = qk_ps.tile([128, 512], f32, tag="qk", name="qkv3")
                qk_slot(g[:], 2)
                # q2 rows 0:64, k2 rows 64:128 stacked in one bank
                nc.vector.tensor_scalar_add(qT_C[0:64, jsl], g[0:64, :],
                                            bq_sb[0:64, 2:3])
                nc.vector.tensor_scalar_add(kT_C[64:128, jsl], g[64:128, :],
                                            bq_sb[64:128, 2:3])
                # duplicate head-2 q/k into the other 64-partition strip so
                # the scores matmul can alternate strips (operands must share
                # the partition range)
                nc.sync.dma_start(qT_C[64:128, jsl], qT_C[0:64, jsl])
                nc.sync.dma_start(kT_C[0:64, jsl], kT_C[64:128, jsl])

            def mkv(tc_):
                # one accumulation group per psum generation: a start=True
                # in a bank wipes other in-flight groups' pending state
                def gen():
                    g = qk_ps.tile([128, 512], f32, tag="qk",
                                   name=f"qkv4_{tc_}")
                    v_chunk(g[:, 0:192], tc_)
                    v_store(g[:, 0:192], tc_)
                return gen

            # (must_before_tile, cost, closure): gen1/2/3 must run early
            # enough that tile j's scores never wait on them; the v chunks
            # by the start of attention(j)
            gens = [gen1, gen2, gen3] + [mkv(t) for t in range(4)]
            tags = [j - 0.4] * 3 + [j] * 4
            costs = [1280] * 3 + [480] * 4
            return list(zip(tags, costs, gens))

        import os
        KSTAGE = int(os.environ.get("KSTAGE", "5"))

        # chunks whose normalize+transpose ran in iteration j; their output
        # projection is deferred into iteration j+1 so the PE never waits on
        # a transpose DMA at the head of its in-order queue.
        pending_proj = []

        def proj_tasks():
            """Deferred output projection of the chunks queued in
            pending_proj, as filler closures. py psum comes from the qk pool
            (its consumers depend only on their own producers, so
            interleaving cannot deadlock the PE FIFO)."""
            if KSTAGE < 5:
                pending_proj.clear()
                return []
            tasks = []
            for m, aoT_ab, aoT_bc in pending_proj:
                def mk(m, aoT_ab, aoT_bc):
                    state = {}

                    def genA():
                        state["y"] = y_pool.tile([128, C], bf16, tag="y",
                                                 name="ysb")
                        py = qk_ps.tile([128, 512], f32, tag="qk",
                                        name="py")[:, 0:384]
                        nc.tensor.matmul(py, aoT_ab[:], wpA_sb[:, 0:384],
                                         start=True, stop=False,
                                         skip_group_check=True)
                        nc.tensor.matmul(py, aoT_bc[64:128, :],
                                         wpC_sb[64:128, 0:384],
                                         start=False, stop=True,
                                         skip_group_check=True)
                        nc.vector.tensor_copy(state["y"][:, 0:384], py)

                    def genB():
                        py = qk_ps.tile([128, 512], f32, tag="qk",
                                        name="py")[:, 0:384]
                        nc.tensor.matmul(py, aoT_ab[:], wpA_sb[:, 384:768],
                                         start=True, stop=False,
                                         skip_group_check=True)
                        nc.tensor.matmul(py, aoT_bc[64:128, :],
                                         wpC_sb[64:128, 384:768],
                                         start=False, stop=True,
                                         skip_group_check=True)
                        nc.vector.tensor_copy(state["y"][:, 384:768], py)
                        nc.sync.dma_start(y_d[m * 128:(m + 1) * 128, :],
                                          state["y"][:])

                    return [(320, genA), (320, genB)]
                tasks.extend(mk(m, aoT_ab, aoT_bc))
            pending_proj.clear()
            return tasks

        def emit_attention(j, filler):
            """scores -> exp -> mask -> att@v -> normalize -> transpose for
            q-tile j (heads 0,1 via AB tiles; head 2 via C). att@v for a
            round is emitted one round late so its exp/mask dependencies are
            already satisfied when the PE reaches it. One filler closure
            (qkv of j+1 / deferred proj of j-1) is emitted before each
            round's scores so the PE has independent work while a score
            matmul waits on its psum slot."""
            nk = 4 * j + 4
            # force any work that must precede this tile (its own qkv gens)
            while filler and filler[0][0] <= j:
                filler.popleft()[2]()
            debt = [0.0]

            def pop_filler(deficit):
                debt[0] += deficit
                while filler and debt[0] >= filler[0][1]:
                    tile_req, cost, fn = filler.popleft()
                    fn()
                    debt[0] -= cost

            # att accumulators: 12 of [128, 65] packed into two banks.
            # Interleaved matmul groups share each bank, so no matmul may use
            # start=True (it marks the whole 2KB zero-region pending and
            # corrupts the other groups); the banks are zeroed by DVE memset
            # instead and every attv accumulates.
            accA = at_ps.tile([128, 512], f32, tag="acc", name="accA")
            accB = at_ps.tile([128, 512], f32, tag="acc", name="accB")
            nc.vector.memset(accA[:, 0:455], 0.0)
            nc.vector.memset(accB[:, 0:325], 0.0)

            def acc_ap(h, qc):
                i = h * 4 + qc
                if i < 7:
                    return accA[:, i * 65:(i + 1) * 65]
                return accB[:, (i - 7) * 65:(i - 6) * 65]

            def attv(eb, bank, ki, h):
                if KSTAGE < 3:
                    return
                r = ki - 4 * j
                for qc in range(max(r, 0), 4):
                    nc.tensor.matmul(
                        acc_ap(h, qc),
                        eb[:, bank * 512 + qc * 128:bank * 512 + qc * 128 + 128],
                        v_aug[:, ki * 195 + 65 * h:ki * 195 + 65 * h + 65],
                        start=False, stop=(ki == 4 * j + qc),
                        skip_group_check=True)

            def exp_mask(pr, eb, bank, ki, single=True):
                """exp (trimmed to the valid q-range) + causal mask for one
                512-col bank; with single=False the caller batches the exp."""
                r = ki - 4 * j
                t0 = 128 * max(r, 0)
                if single:
                    nc.scalar.activation(
                        eb[:, bank * 512 + t0:(bank + 1) * 512],
                        pr[:, bank * 512 + t0:(bank + 1) * 512],
                        AF.Exp, scale=0.125)
                if r >= 0:
                    # only the exact-diagonal 128-col block needs masking;
                    # everything past it is strictly below the diagonal
                    sl = eb[:, bank * 512 + t0:bank * 512 + t0 + 128]
                    nc.gpsimd.affine_select(
                        sl, sl, pattern=[[1, 128]],
                        compare_op=mybir.AluOpType.is_ge, fill=0.0,
                        base=0, channel_multiplier=-1)

            # --- heads 0,1: one ki per round, 2 psum banks ---
            from collections import deque as _dq
            pend = _dq()  # (eb, ki) of recent rounds; attv runs 2 late
            for ki in range(nk):
                r = ki - 4 * j  # >=0 on the diagonal band
                t0 = 128 * max(r, 0)  # valid q-range start within the tile
                act = 2 * (512 - t0) * 0.833 + (370 if t0 else 185)
                pe = 2 * (512 - t0) * 0.417 + 240
                pop_filler((act - pe) * 0.55)
                ksl = bass.ts(ki, 128)
                pr = sc_ps.tile([128, 1024], f32, tag="sc", name="sc")
                for hh in (0, 1):
                    r0 = 64 * hh
                    nc.tensor.matmul(
                        pr[:, hh * 512 + t0:(hh + 1) * 512],
                        kT_AB[r0:r0 + 64, ksl],
                        qT_AB[r0:r0 + 64, j * 512 + t0:(j + 1) * 512],
                        start=True, stop=True)
                eb = eb_pool.tile([128, 1024], bf16, tag="eb", name="eb")
                if t0 == 0:
                    nc.scalar.activation(eb[:], pr[:], AF.Exp, scale=0.125)
                else:
                    src3 = pr[:].rearrange("p (b q) -> p b q",
                                           q=512)[:, :, t0:512]
                    dst3 = eb[:].rearrange("p (b q) -> p b q",
                                           q=512)[:, :, t0:512]
                    nc.scalar.activation(dst3, src3, AF.Exp, scale=0.125)
                exp_mask(pr, eb, 0, ki, single=False)
                exp_mask(pr, eb, 1, ki, single=False)
                if dbg and j == 0 and ki == int(__import__("os").environ.get("KEB", "0")):
                    nc.sync.dma_start(dbg_out["d_eb"], eb[:])
                pend.append((eb, ki))
                if len(pend) > 2:
                    peb, pki = pend.popleft()
                    attv(peb, 0, pki, 0)
                    attv(peb, 1, pki, 1)
            while pend:
                peb, pki = pend.popleft()
                attv(peb, 0, pki, 0)
                attv(peb, 1, pki, 1)

            # --- head 2: two ki per round using the duplicated C strips ---
            pend = _dq()
            while filler and filler[0][0] <= j + 0.6:
                filler.popleft()[2]()
            for g0 in range(0, nk, 2):
                pop_filler(100)
                pr = sc_ps.tile([128, 1024], f32, tag="sc", name="sc")
                eb = eb_pool.tile([128, 1024], bf16, tag="eb", name="eb")
                diag = g0 + 1 >= 4 * j
                for idx, ki in enumerate((g0, g0 + 1)):
                    r = ki - 4 * j
                    t0 = 128 * max(r, 0)
                    ksl = bass.ts(ki, 128)
                    r0 = 64 * (idx % 2)
                    nc.tensor.matmul(
                        pr[:, idx * 512 + t0:(idx + 1) * 512],
                        kT_C[r0:r0 + 64, ksl],
                        qT_C[r0:r0 + 64, j * 512 + t0:(j + 1) * 512],
                        start=True, stop=True)
                if diag:
                    for idx, ki in enumerate((g0, g0 + 1)):
                        exp_mask(pr, eb, idx, ki)
                else:
                    nc.scalar.activation(eb[:], pr[:], AF.Exp, scale=0.125)
                pend.append((eb, g0))
                if len(pend) > 2:
                    peb, pg0 = pend.popleft()
                    attv(peb, 0, pg0, 2)
                    attv(peb, 1, pg0 + 1, 2)
            while pend:
                peb, pg0 = pend.popleft()
                attv(peb, 0, pg0, 2)
                attv(peb, 1, pg0 + 1, 2)

            # --- normalize + transpose per 128-chunk; projection deferred.
            # Raw psum->sbuf copies release the acc banks quickly so
            # attv(j+1) is not gated on the full normalization chain. ---
            accs = nrm.tile([128, 780], f32, tag="accs", name="accs")
            nc.vector.tensor_copy(accs[:, 0:455], accA[:, 0:455])
            nc.vector.tensor_copy(accs[:, 455:780], accB[:, 0:325])

            def acc_sb(h, qc):
                i = h * 4 + qc
                return accs[:, i * 65:(i + 1) * 65]

            if dbg and j == 0:
                nc.sync.dma_start(dbg_out["d_accs"], accs[:])

            for qc in range(4 if KSTAGE >= 4 else 0):
                pop_filler(400)
                m = 4 * j + qc
                ao = ao_pool.tile([128, HPC * D], bf16, tag="ao", name="ao")
                for h in range(HPC):
                    a = acc_sb(h, qc)
                    rcp = nrm.tile([128, 1], f32, tag="rcp", name="rcp")
                    nc.vector.reciprocal_approx_fast(out=rcp[:],
                                                     in_=a[:, 64:65])
                    nc.vector.tensor_scalar_mul(
                        ao[:, h * 64:(h + 1) * 64], a[:, 0:64], rcp[:])
                aoT_ab = aoT_pool.tile([128, 128], bf16, tag="tab", name="tab")
                aoT_bc = aoT_pool.tile([128, 128], bf16, tag="tbc", name="tbc")
                nc.sync.dma_start_transpose(aoT_ab[:], ao[:, 0:128])
                nc.sync.dma_start_transpose(aoT_bc[:], ao[:, 64:192])
                if dbg and j == 0 and qc == 0:
                    nc.sync.dma_start(dbg_out["d_ao"], ao[:])
                    nc.sync.dma_start(dbg_out["d_aotab"], aoT_ab[:])
                    nc.sync.dma_start(dbg_out["d_aotbc"], aoT_bc[:])
                pending_proj.append((m, aoT_ab, aoT_bc))
                if j == NT - 1 and KSTAGE >= 5:
                    # last tile: flush inline on the now-idle sc banks (two
                    # independent banks per chunk) so the projection chain
                    # does not serialize through the 2-slot qk pool
                    for m2, tab, tbc in pending_proj:
                        y_sb = y_pool.tile([128, C], bf16, tag="y",
                                           name="ysb2")
                        pw = sc_ps.tile([128, 1024], f32, tag="sc",
                                        name="pyw")
                        for ns in range(2):
                            py = pw[:, ns * 512:ns * 512 + 384]
                            nc.tensor.matmul(py, tab[:],
                                             wpA_sb[:, ns * 384:(ns + 1) * 384],
                                             start=True, stop=False,
                                             skip_group_check=True)
                            nc.tensor.matmul(py, tbc[64:128, :],
                                             wpC_sb[64:128,
                                                    ns * 384:(ns + 1) * 384],
                                             start=False, stop=True,
                                             skip_group_check=True)
                            nc.vector.tensor_copy(
                                y_sb[:, ns * 384:(ns + 1) * 384], py)
                        nc.sync.dma_start(y_d[m2 * 128:(m2 + 1) * 128, :],
                                          y_sb[:])
                    pending_proj.clear()

        from collections import deque
        filler = deque()  # entries: (must_before_tile, cost_ns, closure)
        for _, _, gen in qkv_tasks(0):
            gen()
        # tile 1's q/k generations run up front too: tile 0's attention is
        # too small to host them as filler without starving the ACT engine
        t1 = qkv_tasks(1)
        for _, _, gen in t1[:3]:
            gen()
        for j in range(NT):
            if j == 0:
                filler.extend(t1[3:])
            elif j + 1 < NT:
                filler.extend(qkv_tasks(j + 1))
            filler.extend((j + 2, c, fn) for c, fn in proj_tasks())
            if KSTAGE >= 2:
                emit_attention(j, filler)
            else:
                while filler:
                    filler.popleft()[2]()
        while filler:
            filler.popleft()[2]()
        for _, t in proj_tasks():
            t()
        if dbg:
            nc.sync.dma_start(dbg_out["d_qAB"], qT_AB[:])
            nc.sync.dma_start(dbg_out["d_kAB"], kT_AB[:])
            nc.sync.dma_start(dbg_out["d_qC"], qT_C[:])
            nc.sync.dma_start(dbg_out["d_kC"], kT_C[:])
            nc.sync.dma_start(dbg_out["d_vaug"], v_aug[:])

    nc.compile()
    return nc


_NC_CACHE = {}


def _get_nc(T):
    if T not in _NC_CACHE:
        _NC_CACHE[T] = build_nc(T)
    return _NC_CACHE[T]


def make_core_inputs(x, W_attn, b_attn, W_proj):
    """Host-side prep: per-core input dicts (see module docstring)."""
    B, T, _ = x.shape
    xts = [np.ascontiguousarray(x[b].T) for b in range(B)]
    in_maps = []
    for core in range(N_CORES):
        b = core // (N_CORES // B)
        h0 = HPC * (core % (N_CORES // B))
        ccols = slice(h0 * D, (h0 + 2) * D)      # first two heads
        c2 = slice((h0 + 2) * D, (h0 + 3) * D)   # third head
        # reference splits qkv as (k, q, v): k cols 0:C, q cols C:2C, v 2C:3C
        q01 = W_attn[:, C:2 * C][:, ccols]
        k01 = W_attn[:, 0:C][:, ccols]
        q2 = W_attn[:, C:2 * C][:, c2]
        k2 = W_attn[:, 0:C][:, c2]
        wq = np.ascontiguousarray(
            np.concatenate([q01, k01, q2, k2], axis=1))
        wv = np.ascontiguousarray(
            W_attn[:, 2 * C:3 * C][:, h0 * D:(h0 + HPC) * D])
        bq = np.zeros((128, 3), np.float32)
        bq[:, 0] = b_attn[C:2 * C][ccols]
        bq[:, 1] = b_attn[0:C][ccols]
        bq[0:64, 2] = b_attn[C:2 * C][c2]
        bq[64:128, 2] = b_attn[0:C][c2]
        wp = np.ascontiguousarray(W_proj[h0 * D:(h0 + HPC) * D, :])
        def pmaj(a):
            # [CK*128, f] -> [128, CK*f]
            f = a.shape[1]
            return np.ascontiguousarray(
                a.reshape(-1, 128, f).transpose(1, 0, 2).reshape(128, -1))

        in_maps.append({
            "xt": to_bf16(pmaj(xts[b])),
            "wq": to_bf16(pmaj(wq)),
            "wv": to_bf16(pmaj(wv)),
            "bq": bq,
            "wp": to_bf16(wp),
        })
    return in_maps


def to_bf16(a):
    import ml_dtypes
    return np.ascontiguousarray(a.astype(ml_dtypes.bfloat16))


def kernel(x, W_attn, b_attn, W_proj, b_proj):
    x = np.asarray(x, dtype=np.float32)
    W_attn = np.asarray(W_attn, dtype=np.float32)
    b_attn = np.asarray(b_attn, dtype=np.float32)
    W_proj = np.asarray(W_proj, dtype=np.float32)
    b_proj = np.asarray(b_proj, dtype=np.float32)
    B, T, _ = x.shape

    nc = _get_nc(T)
    in_maps = make_core_inputs(x, W_attn, b_attn, W_proj)
    res = None
    for attempt in range(3):
        try:
            res = run_bass_kernel_spmd(nc, in_maps, list(range(N_CORES)))
            break
        except Exception:
            # transient NRT_EXEC_UNIT_UNRECOVERABLE has been observed once
            # after a prior crashed process; a retry succeeds
            if attempt == 2:
                raise
    global LAST_RUN
    LAST_RUN = res

    # the v bias contributes b_v @ W_proj to every token (softmax rows sum
    # to 1), folded here instead of inside the kernel
    b_eff = b_proj + b_attn[2 * C:3 * C] @ W_proj

    gpb = N_CORES // B
    out = np.empty((B, T, C), np.float32)
    for b in range(B):
        acc = res.results[b * gpb]["y"].astype(np.float32)
        for g in range(1, gpb):
            acc = acc + res.results[b * gpb + g]["y"]
        out[b] = acc + b_eff[None, :]
    return out


# revision 46
# speedup vs baseline: 1.0403x; 1.0010x over previous
"""Causal self-attention (B=2, T=4096, C=768, H=12) on 8 TRN2 NeuronCores.

Sharding: batch x head-group. Core c handles batch b=c//4 and heads
h0..h0+2 where h0 = 3*(c%4). Each core computes the qkv projection for
its 3 heads, full causal attention, and a partial output projection; the
host sums the 4 partials per batch and adds the (v-bias-folded)
projection bias.

All matmul operands are bf16 (1 cycle/row on the PE at any moving size).
q/k live transposed ([D, T]) feeding the scores matmul; v is computed in
natural token-major layout and packed into v_aug [k, 65*3] with a ones
column per head so the softmax denominator falls out of att@v as output
column 64. att@v runs in [q, d] orientation (stationary eb^T chunk
[k,128q], moving v_aug [k,65]) so each 128x128 block costs only 65 PE
cycles and the denominator lands as a per-partition column, normalized
with reciprocal + tensor_scalar (no partition broadcast). The normalized
ao [tok, hd] is flipped to [hd, tok] with DMA-engine xbar transposes and
fed to the output projection as the stationary operand.

Causality: scores/exp are only computed for k-tiles at or below the
diagonal, trimmed to the valid q-range on the diagonal band; the
remaining intra-block mask is applied in-place on eb by gpsimd
affine_select. The v bias is algebraically folded into the host-side
projection bias (softmax weights sum to 1).
"""

import sys

for _p in ("/opt/trn_rl_repo",):
    if _p not in sys.path:
        sys.path.insert(0, _p)

from contextlib import ExitStack

import numpy as np

import concourse.bass as bass  # noqa: F401
import concourse.mybir as mybir
import concourse.tile as tile
from concourse import bacc
from concourse.bass_utils import run_bass_kernel_spmd

f32 = mybir.dt.float32
bf16 = mybir.dt.bfloat16
AF = mybir.ActivationFunctionType

C = 768
D = 64
N_HEAD = 12
HPC = 3  # heads per core
N_CORES = 8

# wq column slots: q01 | k01 | (q2 stacked over k2)
QK_SLOTS = [(0, 128), (128, 256), (256, 384)]


def build_nc(T):
    NT = T // 512  # q tiles
    KT = T // 128  # k tiles / token chunks
    CK = C // 128  # contraction chunks

    nc = bacc.Bacc("TRN2", target_bir_lowering=False, debug=False,
                   num_devices=N_CORES)
    # merged partition-major layouts: [p, c*cols + f] so each load is ONE
    # DMA instead of CK of them (HWDGE is a single serial device)
    xt_d = nc.dram_tensor("xt", [128, CK * T], bf16, kind="ExternalInput").ap()
    wq_d = nc.dram_tensor("wq", [128, CK * 384], bf16,
                          kind="ExternalInput").ap()
    wv_d = nc.dram_tensor("wv", [128, CK * HPC * D], bf16,
                          kind="ExternalInput").ap()
    bq_d = nc.dram_tensor("bq", [128, 3], f32, kind="ExternalInput").ap()
    wp_d = nc.dram_tensor("wp", [HPC * D, C], bf16, kind="ExternalInput").ap()
    y_d = nc.dram_tensor("y", [T, C], bf16, kind="ExternalOutput").ap()
    import os
    dbg = os.environ.get("KDBG") == "1"
    dbg_out = {}
    if dbg:
        for nm, shp, dt in [("d_qAB", [128, T], bf16), ("d_kAB", [128, T], bf16),
                            ("d_qC", [128, T], bf16), ("d_kC", [128, T], bf16),
                            ("d_vaug", [128, 32 * 195], bf16),
                            ("d_eb", [128, 1024], bf16),
                            ("d_accs", [128, 780], f32),
                            ("d_ao", [128, 192], bf16),
                            ("d_aotab", [128, 128], bf16),
                            ("d_aotbc", [128, 128], bf16)]:
            dbg_out[nm] = nc.dram_tensor(nm, shp, dt, kind="ExternalOutput").ap()

    with tile.TileContext(nc) as tc, ExitStack() as ctx:
        sb = ctx.enter_context(tc.tile_pool(name="sb", bufs=1))

        # persistent tensors
        bq_sb = sb.tile([128, 3], f32, tag="bq")
        qT_AB = sb.tile([128, T], bf16, tag="qAB")
        kT_AB = sb.tile([128, T], bf16, tag="kAB")
        qT_C = sb.tile([128, T], bf16, tag="qC")
        kT_C = sb.tile([128, T], bf16, tag="kC")
        v_aug = sb.tile([128, KT * 195], bf16, tag="vaug")
        wq_sb2 = sb.tile([128, CK * 384], bf16, tag="wq")
        wv_sb2 = sb.tile([128, CK * HPC * D], bf16, tag="wv")
        wq_sb = [wq_sb2[:, c * 384:(c + 1) * 384] for c in range(CK)]
        wv_sb = [wv_sb2[:, c * HPC * D:(c + 1) * HPC * D] for c in range(CK)]
        wpA_sb = sb.tile([128, C], bf16, tag="wpA")   # heads 0,1
        # head 2 lives at partitions 64:128 to match aoT_bc's h2 strip
        wpC_sb = sb.tile([128, C], bf16, tag="wpC")

        nc.sync.dma_start(bq_sb[:], bq_d)
        # ones columns of v_aug (cols 64,129,194 mod 195) come from this
        # blanket fill; the v copies below overwrite the 64-col data slices.
        nc.gpsimd.memset(v_aug[:], 1.0)

        # PE warm-up: ~3.5us of dummy matmuls bridge the initial xt DMA
        # latency so the p-state ramp completes before the first real matmul
        warm = sb.tile([128, 512], bf16, tag="warm")
        nc.vector.memset(warm[:], 0.5)
        with tc.tile_pool(name="wups", bufs=1, space="PSUM") as wu_ps:
            wu = wu_ps.tile([128, 512], f32, tag="wu")
            for _ in range(12):
                nc.tensor.matmul(wu[:], warm[:, 0:128], warm[:],
                                 start=True, stop=True,
                                 skip_group_check=True)

        # xt strips: whole [128, T] rows per contraction chunk; the j=0
        # slice loads first so the pipeline can start early.
        xt_sb2 = sb.tile([128, CK * T], bf16, tag="xts")
        xt_sb = [xt_sb2[:, c * T:(c + 1) * T] for c in range(CK)]
        xt3 = xt_sb2[:].rearrange("p (c t) -> p c t", t=T)
        xt3_d = xt_d.rearrange("p (c t) -> p c t", t=T)

        def xt_wave(a, b):
            nc.sync.dma_start(xt3[:, :, a:b], xt3_d[:, :, a:b])

        nc.sync.dma_start(wq_sb2[:], wq_d)
        xt_wave(0, 512)
        xt_wave(512, 1024)
        nc.sync.dma_start(wv_sb2[:], wv_d)
        nc.sync.dma_start(wpA_sb[:], wp_d[0:128, :])
        nc.sync.dma_start(wpC_sb[64:128, :], wp_d[128:192, :])
        for half in range(1, 4):
            xt_wave(half * 1024, (half + 1) * 1024)
        # PSUM budget (8 banks): sc 2x2 + acc 2 + qkv 2. The acc slots
        # also serve the deferred projection (they are idle between norm(j)
        # and attv(j+1)); qkv rotates its 5 generations through 2 banks.
        sc_ps = ctx.enter_context(
            tc.tile_pool(name="scps", bufs=2, space="PSUM"))
        at_ps = ctx.enter_context(
            tc.tile_pool(name="atps", bufs=2, space="PSUM"))
        qk_ps = ctx.enter_context(
            tc.tile_pool(name="qkps", bufs=2, space="PSUM"))
        eb_pool = ctx.enter_context(tc.tile_pool(name="ebp", bufs=12))
        ao_pool = ctx.enter_context(tc.tile_pool(name="aop", bufs=6))
        aoT_pool = ctx.enter_context(tc.tile_pool(name="aotp", bufs=6))
        y_pool = ctx.enter_context(tc.tile_pool(name="yp", bufs=4))
        nrm = ctx.enter_context(tc.tile_pool(name="nrm", bufs=4))

        def qkv_tasks(j):
            """qkv projection for q-tile j as a list of filler closures (one
            per psum generation) so the PE work can interleave between
            attention rounds of the previous tile."""
            jsl = bass.ts(j, 512)
            xt_t = [xt_sb[c][:, jsl] for c in range(CK)]

            def qk_slot(ps_reg, s):
                c0, c1 = QK_SLOTS[s]
                for c in range(CK):
                    nc.tensor.matmul(ps_reg, wq_sb[c][:, c0:c1], xt_t[c],
                                     start=(c == 0), stop=(c == CK - 1),
                                     skip_group_check=True)

            def v_chunk(ps_reg, tc_):
                for c in range(CK):
                    nc.tensor.matmul(ps_reg,
                                     xt_t[c][:, tc_ * 128:(tc_ + 1) * 128],
                                     wv_sb[c],
                                     start=(c == 0), stop=(c == CK - 1),
                                     skip_group_check=True)

            def v_store(ps_reg, tc_):
                base = (4 * j + tc_) * 195
                dst = v_aug[:, base:base + 195].rearrange(
                    "p (h c) -> p h c", c=65)[:, :, 0:64]
                nc.vector.tensor_copy(
                    dst, ps_reg.rearrange("p (h c) -> p h c", c=64))

            def gen1():
                g = qk_ps.tile([128, 512], f32, tag="qk", name="qkv1")
                qk_slot(g[:], 0)
                nc.vector.tensor_scalar_add(qT_AB[:, jsl], g[:],
                                            bq_sb[:, 0:1])

            def gen2():
                g = qk_ps.tile([128, 512], f32, tag="qk", name="qkv2")
                qk_slot(g[:], 1)
                nc.vector.tensor_scalar_add(kT_AB[:, jsl], g[:],
                                            bq_sb[:, 1:2])

            def gen3():
                g = qk_ps.tile([128, 512], f32, tag="qk", name="qkv3")
                qk_slot(g[:], 2)
                # q2 rows 0:64, k2 rows 64:128 stacked in one bank
                nc.vector.tensor_scalar_add(qT_C[0:64, jsl], g[0:64, :],
                                            bq_sb[0:64, 2:3])
                nc.vector.tensor_scalar_add(kT_C[64:128, jsl], g[64:128, :],
                                            bq_sb[64:128, 2:3])
                # duplicate head-2 q/k into the other 64-partition strip so
                # the scores matmul can alternate strips (operands must share
                # the partition range)
                nc.sync.dma_start(qT_C[64:128, jsl], qT_C[0:64, jsl])
                nc.sync.dma_start(kT_C[0:64, jsl], kT_C[64:128, jsl])

            def mkv(tc_):
                # one accumulation group per psum generation: a start=True
                # in a bank wipes other in-flight groups' pending state
                def gen():
                    g = qk_ps.tile([128, 512], f32, tag="qk",
                                   name=f"qkv4_{tc_}")
                    v_chunk(g[:, 0:192], tc_)
                    v_store(g[:, 0:192], tc_)
                return gen

            # (must_before_tile, cost, closure): gen1/2/3 must run early
            # enough that tile j's scores never wait on them; the v chunks
            # by the start of attention(j)
            gens = [gen1, gen2, gen3] + [mkv(t) for t in range(4)]
            tags = [j - 0.4] * 3 + [j] * 4
            costs = [1280] * 3 + [480] * 4
            return list(zip(tags, costs, gens))

        import os
        KSTAGE = int(os.environ.get("KSTAGE", "5"))

        # chunks whose normalize+transpose ran in iteration j; their output
        # projection is deferred into iteration j+1 so the PE never waits on
        # a transpose DMA at the head of its in-order queue.
        pending_proj = []

        def proj_tasks():
            """Deferred output projection of the chunks queued in
            pending_proj, as filler closures. py psum comes from the qk pool
            (its consumers depend only on their own producers, so
            interleaving cannot deadlock the PE FIFO)."""
            if KSTAGE < 5:
                pending_proj.clear()
                return []
            tasks = []
            for m, aoT_ab, aoT_bc in pending_proj:
                def mk(m, aoT_ab, aoT_bc):
                    state = {}

                    def genA():
                        state["y"] = y_pool.tile([128, C], bf16, tag="y",
                                                 name="ysb")
                        py = qk_ps.tile([128, 512], f32, tag="qk",
                                        name="py")[:, 0:384]
                        nc.tensor.matmul(py, aoT_ab[:], wpA_sb[:, 0:384],
                                         start=True, stop=False,
                                         skip_group_check=True)
                        nc.tensor.matmul(py, aoT_bc[64:128, :],
                                         wpC_sb[64:128, 0:384],
                                         start=False, stop=True,
                                         skip_group_check=True)
                        nc.vector.tensor_copy(state["y"][:, 0:384], py)

                    def genB():
                        py = qk_ps.tile([128, 512], f32, tag="qk",
                                        name="py")[:, 0:384]
                        nc.tensor.matmul(py, aoT_ab[:], wpA_sb[:, 384:768],
                                         start=True, stop=False,
                                         skip_group_check=True)
                        nc.tensor.matmul(py, aoT_bc[64:128, :],
                                         wpC_sb[64:128, 384:768],
                                         start=False, stop=True,
                                         skip_group_check=True)
                        nc.vector.tensor_copy(state["y"][:, 384:768], py)
                        nc.sync.dma_start(y_d[m * 128:(m + 1) * 128, :],
                                          state["y"][:])

                    return [(320, genA), (320, genB)]
                tasks.extend(mk(m, aoT_ab, aoT_bc))
            pending_proj.clear()
            return tasks

        def emit_attention(j, filler):
            """scores -> exp -> mask -> att@v -> normalize -> transpose for
            q-tile j (heads 0,1 via AB tiles; head 2 via C). att@v for a
            round is emitted one round late so its exp/mask dependencies are
            already satisfied when the PE reaches it. One filler closure
            (qkv of j+1 / deferred proj of j-1) is emitted before each
            round's scores so the PE has independent work while a score
            matmul waits on its psum slot."""
            nk = 4 * j + 4
            # force any work that must precede this tile (its own qkv gens)
            while filler and filler[0][0] <= j:
                filler.popleft()[2]()
            debt = [0.0]

            def pop_filler(deficit):
                debt[0] += deficit
                while filler and debt[0] >= filler[0][1]:
                    tile_req, cost, fn = filler.popleft()
                    fn()
                    debt[0] -= cost

            # att accumulators: 12 of [128, 65] packed into two banks.
            # Interleaved matmul groups share each bank, so no matmul may use
            # start=True (it marks the whole 2KB zero-region pending and
            # corrupts the other groups); the banks are zeroed by DVE memset
            # instead and every attv accumulates.
            accA = at_ps.tile([128, 512], f32, tag="acc", name="accA")
            accB = at_ps.tile([128, 512], f32, tag="acc", name="accB")
            nc.vector.memset(accA[:, 0:455], 0.0)
            nc.vector.memset(accB[:, 0:325], 0.0)

            def acc_ap(h, qc):
                i = h * 4 + qc
                if i < 7:
                    return accA[:, i * 65:(i + 1) * 65]
                return accB[:, (i - 7) * 65:(i - 6) * 65]

            def attv(eb, bank, ki, h):
                if KSTAGE < 3:
                    return
                r = ki - 4 * j
                for qc in range(max(r, 0), 4):
                    nc.tensor.matmul(
                        acc_ap(h, qc),
                        eb[:, bank * 512 + qc * 128:bank * 512 + qc * 128 + 128],
                        v_aug[:, ki * 195 + 65 * h:ki * 195 + 65 * h + 65],
                        start=False, stop=(ki == 4 * j + qc),
                        skip_group_check=True)

            def exp_mask(pr, eb, bank, ki, single=True):
                """exp (trimmed to the valid q-range) + causal mask for one
                512-col bank; with single=False the caller batches the exp."""
                r = ki - 4 * j
                t0 = 128 * max(r, 0)
                if single:
                    nc.scalar.activation(
                        eb[:, bank * 512 + t0:(bank + 1) * 512],
                        pr[:, bank * 512 + t0:(bank + 1) * 512],
                        AF.Exp, scale=0.125)
                if r >= 0:
                    # only the exact-diagonal 128-col block needs masking;
                    # everything past it is strictly below the diagonal
                    sl = eb[:, bank * 512 + t0:bank * 512 + t0 + 128]
                    nc.gpsimd.affine_select(
                        sl, sl, pattern=[[1, 128]],
                        compare_op=mybir.AluOpType.is_ge, fill=0.0,
                        base=0, channel_multiplier=-1)

            # --- heads 0,1: one ki per round, 2 psum banks ---
            from collections import deque as _dq
            pend = _dq()  # (eb, ki) of recent rounds; attv runs 2 late
            for ki in range(nk):
                r = ki - 4 * j  # >=0 on the diagonal band
                t0 = 128 * max(r, 0)  # valid q-range start within the tile
                act = 2 * (512 - t0) * 0.833 + (370 if t0 else 185)
                pe = 2 * (512 - t0) * 0.417 + 240
                pop_filler((act - pe) * 0.55)
                ksl = bass.ts(ki, 128)
                pr = sc_ps.tile([128, 1024], f32, tag="sc", name="sc")
                for hh in (0, 1):
                    r0 = 64 * hh
                    nc.tensor.matmul(
                        pr[:, hh * 512 + t0:(hh + 1) * 512],
                        kT_AB[r0:r0 + 64, ksl],
                        qT_AB[r0:r0 + 64, j * 512 + t0:(j + 1) * 512],
                        start=True, stop=True)
                eb = eb_pool.tile([128, 1024], bf16, tag="eb", name="eb")
                if t0 == 0:
                    nc.scalar.activation(eb[:], pr[:], AF.Exp, scale=0.125)
                else:
                    src3 = pr[:].rearrange("p (b q) -> p b q",
                                           q=512)[:, :, t0:512]
                    dst3 = eb[:].rearrange("p (b q) -> p b q",
                                           q=512)[:, :, t0:512]
                    nc.scalar.activation(dst3, src3, AF.Exp, scale=0.125)
                exp_mask(pr, eb, 0, ki, single=False)
                exp_mask(pr, eb, 1, ki, single=False)
                if dbg and j == 0 and ki == int(__import__("os").environ.get("KEB", "0")):
                    nc.sync.dma_start(dbg_out["d_eb"], eb[:])
                pend.append((eb, ki))
                if len(pend) > 2:
                    peb, pki = pend.popleft()
                    attv(peb, 0, pki, 0)
                    attv(peb, 1, pki, 1)
            while pend:
                peb, pki = pend.popleft()
                attv(peb, 0, pki, 0)
                attv(peb, 1, pki, 1)

            # --- head 2: two ki per round using the duplicated C strips ---
            pend = _dq()
            while filler and filler[0][0] <= j + 0.6:
                filler.popleft()[2]()
            for g0 in range(0, nk, 2):
                pop_filler(100)
                pr = sc_ps.tile([128, 1024], f32, tag="sc", name="sc")
                eb = eb_pool.tile([128, 1024], bf16, tag="eb", name="eb")
                diag = g0 + 1 >= 4 * j
                for idx, ki in enumerate((g0, g0 + 1)):
                    r = ki - 4 * j
                    t0 = 128 * max(r, 0)
                    ksl = bass.ts(ki, 128)
                    r0 = 64 * (idx % 2)
                    nc.tensor.matmul(
                        pr[:, idx * 512 + t0:(idx + 1) * 512],
                        kT_C[r0:r0 + 64, ksl],
                        qT_C[r0:r0 + 64, j * 512 + t0:(j + 1) * 512],
                        start=True, stop=True)
                if diag:
                    for idx, ki in enumerate((g0, g0 + 1)):
                        exp_mask(pr, eb, idx, ki)
                else:
                    nc.scalar.activation(eb[:], pr[:], AF.Exp, scale=0.125)
                pend.append((eb, g0))
                if len(pend) > 2:
                    peb, pg0 = pend.popleft()
                    attv(peb, 0, pg0, 2)
                    attv(peb, 1, pg0 + 1, 2)
            while pend:
                peb, pg0 = pend.popleft()
                attv(peb, 0, pg0, 2)
                attv(peb, 1, pg0 + 1, 2)

            # --- normalize + transpose per 128-chunk; projection deferred.
            # Raw psum->sbuf copies release the acc banks quickly so
            # attv(j+1) is not gated on the full normalization chain. ---
            accs = nrm.tile([128, 780], f32, tag="accs", name="accs")
            nc.vector.tensor_copy(accs[:, 0:455], accA[:, 0:455])
            nc.vector.tensor_copy(accs[:, 455:780], accB[:, 0:325])

            def acc_sb(h, qc):
                i = h * 4 + qc
                return accs[:, i * 65:(i + 1) * 65]

            if dbg and j == 0:
                nc.sync.dma_start(dbg_out["d_accs"], accs[:])

            # all 12 denominators inverted in ONE strided custom-DVE op
            # (they sit at accs cols 64+65*i) -- keeps the tile-end DVE
            # chain short
            rcp12 = nrm.tile([128, 12], f32, tag="rcp", name="rcp12")
            if KSTAGE >= 4:
                den = accs[:].rearrange("p (i c) -> p i c", c=65)[:, :, 64:65]
                nc.vector.reciprocal_approx_fast(
                    out=rcp12[:].rearrange("p (i c) -> p i c", c=1),
                    in_=den)
            for qc in range(4 if KSTAGE >= 4 else 0):
                pop_filler(400)
                m = 4 * j + qc
                ao = ao_pool.tile([128, HPC * D], bf16, tag="ao", name="ao")
                for h in range(HPC):
                    a = acc_sb(h, qc)
                    i = h * 4 + qc
                    nc.vector.tensor_scalar_mul(
                        ao[:, h * 64:(h + 1) * 64], a[:, 0:64],
                        rcp12[:, i:i + 1])
                aoT_ab = aoT_pool.tile([128, 128], bf16, tag="tab", name="tab")
                aoT_bc = aoT_pool.tile([128, 128], bf16, tag="tbc", name="tbc")
                nc.sync.dma_start_transpose(aoT_ab[:], ao[:, 0:128])
                nc.sync.dma_start_transpose(aoT_bc[:], ao[:, 64:192])
                if dbg and j == 0 and qc == 0:
                    nc.sync.dma_start(dbg_out["d_ao"], ao[:])
                    nc.sync.dma_start(dbg_out["d_aotab"], aoT_ab[:])
                    nc.sync.dma_start(dbg_out["d_aotbc"], aoT_bc[:])
                pending_proj.append((m, aoT_ab, aoT_bc))
                if j == NT - 1 and KSTAGE >= 5:
                    # last tile: flush inline on the now-idle sc banks (two
                    # independent banks per chunk) so the projection chain
                    # does not serialize through the 2-slot qk pool
                    for m2, tab, tbc in pending_proj:
                        y_sb = y_pool.tile([128, C], bf16, tag="y",
                                           name="ysb2")
                        pw = sc_ps.tile([128, 1024], f32, tag="sc",
                                        name="pyw")
                        for ns in range(2):
                            py = pw[:, ns * 512:ns * 512 + 384]
                            nc.tensor.matmul(py, tab[:],
                                             wpA_sb[:, ns * 384:(ns + 1) * 384],
                                             start=True, stop=False,
                                             skip_group_check=True)
                            nc.tensor.matmul(py, tbc[64:128, :],
                                             wpC_sb[64:128,
                                                    ns * 384:(ns + 1) * 384],
                                             start=False, stop=True,
                                             skip_group_check=True)
                            nc.vector.tensor_copy(
                                y_sb[:, ns * 384:(ns + 1) * 384], py)
                        nc.sync.dma_start(y_d[m2 * 128:(m2 + 1) * 128, :],
                                          y_sb[:])
                    pending_proj.clear()

        from collections import deque
        filler = deque()  # entries: (must_before_tile, cost_ns, closure)
        for _, _, gen in qkv_tasks(0):
            gen()
        # tile 1's q/k generations run up front too: tile 0's attention is
        # too small to host them as filler without starving the ACT engine
        t1 = qkv_tasks(1)
        for _, _, gen in t1[:3]:
            gen()
        for j in range(NT):
            if j == 0:
                filler.extend(t1[3:])
            elif j + 1 < NT:
                filler.extend(qkv_tasks(j + 1))
            filler.extend((j + 2, c, fn) for c, fn in proj_tasks())
            if KSTAGE >= 2:
                emit_attention(j, filler)
            else:
                while filler:
                    filler.popleft()[2]()
        while filler:
            filler.popleft()[2]()
        for _, t in proj_tasks():
            t()
        if dbg:
            nc.sync.dma_start(dbg_out["d_qAB"], qT_AB[:])
            nc.sync.dma_start(dbg_out["d_kAB"], kT_AB[:])
            nc.sync.dma_start(dbg_out["d_qC"], qT_C[:])
            nc.sync.dma_start(dbg_out["d_kC"], kT_C[:])
            nc.sync.dma_start(dbg_out["d_vaug"], v_aug[:])

    nc.compile()
    return nc


_NC_CACHE = {}


def _get_nc(T):
    if T not in _NC_CACHE:
        _NC_CACHE[T] = build_nc(T)
    return _NC_CACHE[T]


def make_core_inputs(x, W_attn, b_attn, W_proj):
    """Host-side prep: per-core input dicts (see module docstring)."""
    B, T, _ = x.shape
    xts = [np.ascontiguousarray(x[b].T) for b in range(B)]
    in_maps = []
    for core in range(N_CORES):
        b = core // (N_CORES // B)
        h0 = HPC * (core % (N_CORES // B))
        ccols = slice(h0 * D, (h0 + 2) * D)      # first two heads
        c2 = slice((h0 + 2) * D, (h0 + 3) * D)   # third head
        # reference splits qkv as (k, q, v): k cols 0:C, q cols C:2C, v 2C:3C
        q01 = W_attn[:, C:2 * C][:, ccols]
        k01 = W_attn[:, 0:C][:, ccols]
        q2 = W_attn[:, C:2 * C][:, c2]
        k2 = W_attn[:, 0:C][:, c2]
        wq = np.ascontiguousarray(
            np.concatenate([q01, k01, q2, k2], axis=1))
        wv = np.ascontiguousarray(
            W_attn[:, 2 * C:3 * C][:, h0 * D:(h0 + HPC) * D])
        bq = np.zeros((128, 3), np.float32)
        bq[:, 0] = b_attn[C:2 * C][ccols]
        bq[:, 1] = b_attn[0:C][ccols]
        bq[0:64, 2] = b_attn[C:2 * C][c2]
        bq[64:128, 2] = b_attn[0:C][c2]
        wp = np.ascontiguousarray(W_proj[h0 * D:(h0 + HPC) * D, :])
        def pmaj(a):
            # [CK*128, f] -> [128, CK*f]
            f = a.shape[1]
            return np.ascontiguousarray(
                a.reshape(-1, 128, f).transpose(1, 0, 2).reshape(128, -1))

        in_maps.append({
            "xt": to_bf16(pmaj(xts[b])),
            "wq": to_bf16(pmaj(wq)),
            "wv": to_bf16(pmaj(wv)),
            "bq": bq,
            "wp": to_bf16(wp),
        })
    return in_maps


def to_bf16(a):
    import ml_dtypes
    return np.ascontiguousarray(a.astype(ml_dtypes.bfloat16))


def kernel(x, W_attn, b_attn, W_proj, b_proj):
    x = np.asarray(x, dtype=np.float32)
    W_attn = np.asarray(W_attn, dtype=np.float32)
    b_attn = np.asarray(b_attn, dtype=np.float32)
    W_proj = np.asarray(W_proj, dtype=np.float32)
    b_proj = np.asarray(b_proj, dtype=np.float32)
    B, T, _ = x.shape

    nc = _get_nc(T)
    in_maps = make_core_inputs(x, W_attn, b_attn, W_proj)
    res = None
    for attempt in range(3):
        try:
            res = run_bass_kernel_spmd(nc, in_maps, list(range(N_CORES)))
            break
        except Exception:
            # transient NRT_EXEC_UNIT_UNRECOVERABLE has been observed once
            # after a prior crashed process; a retry succeeds
            if attempt == 2:
                raise
    global LAST_RUN
    LAST_RUN = res

    # the v bias contributes b_v @ W_proj to every token (softmax rows sum
    # to 1), folded here instead of inside the kernel
    b_eff = b_proj + b_attn[2 * C:3 * C] @ W_proj

    gpb = N_CORES // B
    out = np.empty((B, T, C), np.float32)
    for b in range(B):
        acc = res.results[b * gpb]["y"].astype(np.float32)
        for g in range(1, gpb):
            acc = acc + res.results[b * gpb + g]["y"]
        out[b] = acc + b_eff[None, :]
    return out


# revision 51
# speedup vs baseline: 1.0407x; 1.0003x over previous
"""Causal self-attention (B=2, T=4096, C=768, H=12) on 8 TRN2 NeuronCores.

Sharding: batch x head-group. Core c handles batch b=c//4 and heads
h0..h0+2 where h0 = 3*(c%4). Each core computes the qkv projection for
its 3 heads, full causal attention, and a partial output projection; the
host sums the 4 partials per batch and adds the (v-bias-folded)
projection bias.

All matmul operands are bf16 (1 cycle/row on the PE at any moving size).
q/k live transposed ([D, T]) feeding the scores matmul; v is computed in
natural token-major layout and packed into v_aug [k, 65*3] with a ones
column per head so the softmax denominator falls out of att@v as output
column 64. att@v runs in [q, d] orientation (stationary eb^T chunk
[k,128q], moving v_aug [k,65]) so each 128x128 block costs only 65 PE
cycles and the denominator lands as a per-partition column, normalized
with reciprocal + tensor_scalar (no partition broadcast). The normalized
ao [tok, hd] is flipped to [hd, tok] with DMA-engine xbar transposes and
fed to the output projection as the stationary operand.

Causality: scores/exp are only computed for k-tiles at or below the
diagonal, trimmed to the valid q-range on the diagonal band; the
remaining intra-block mask is applied in-place on eb by gpsimd
affine_select. The v bias is algebraically folded into the host-side
projection bias (softmax weights sum to 1).
"""

import sys

for _p in ("/opt/trn_rl_repo",):
    if _p not in sys.path:
        sys.path.insert(0, _p)

from contextlib import ExitStack

import numpy as np

import concourse.bass as bass  # noqa: F401
import concourse.mybir as mybir
import concourse.tile as tile
from concourse import bacc
from concourse.bass_utils import run_bass_kernel_spmd

f32 = mybir.dt.float32
bf16 = mybir.dt.bfloat16
AF = mybir.ActivationFunctionType

C = 768
D = 64
N_HEAD = 12
HPC = 3  # heads per core
N_CORES = 8

# wq column slots: q01 | k01 | (q2 stacked over k2)
QK_SLOTS = [(0, 128), (128, 256), (256, 384)]


def build_nc(T):
    NT = T // 512  # q tiles
    KT = T // 128  # k tiles / token chunks
    CK = C // 128  # contraction chunks

    nc = bacc.Bacc("TRN2", target_bir_lowering=False, debug=False,
                   num_devices=N_CORES)
    # merged partition-major layouts: [p, c*cols + f] so each load is ONE
    # DMA instead of CK of them (HWDGE is a single serial device)
    xt_d = nc.dram_tensor("xt", [128, CK * T], bf16, kind="ExternalInput").ap()
    wq_d = nc.dram_tensor("wq", [128, CK * 384], bf16,
                          kind="ExternalInput").ap()
    wv_d = nc.dram_tensor("wv", [128, CK * HPC * D], bf16,
                          kind="ExternalInput").ap()
    bq_d = nc.dram_tensor("bq", [128, 3], f32, kind="ExternalInput").ap()
    wp_d = nc.dram_tensor("wp", [HPC * D, C], bf16, kind="ExternalInput").ap()
    y_d = nc.dram_tensor("y", [T, C], bf16, kind="ExternalOutput").ap()
    import os
    dbg = os.environ.get("KDBG") == "1"
    dbg_out = {}
    if dbg:
        for nm, shp, dt in [("d_qAB", [128, T], bf16), ("d_kAB", [128, T], bf16),
                            ("d_qC", [128, T], bf16), ("d_kC", [128, T], bf16),
                            ("d_vaug", [128, 32 * 195], bf16),
                            ("d_eb", [128, 1024], bf16),
                            ("d_accs", [128, 780], f32),
                            ("d_ao", [128, 192], bf16),
                            ("d_aotab", [128, 128], bf16),
                            ("d_aotbc", [128, 128], bf16)]:
            dbg_out[nm] = nc.dram_tensor(nm, shp, dt, kind="ExternalOutput").ap()

    with tile.TileContext(nc) as tc, ExitStack() as ctx:
        sb = ctx.enter_context(tc.tile_pool(name="sb", bufs=1))

        # persistent tensors
        bq_sb = sb.tile([128, 3], f32, tag="bq")
        qT_AB = sb.tile([128, T], bf16, tag="qAB")
        kT_AB = sb.tile([128, T], bf16, tag="kAB")
        qT_C = sb.tile([128, T], bf16, tag="qC")
        kT_C = sb.tile([128, T], bf16, tag="kC")
        v_aug = sb.tile([128, KT * 195], bf16, tag="vaug")
        wq_sb2 = sb.tile([128, CK * 384], bf16, tag="wq")
        wv_sb2 = sb.tile([128, CK * HPC * D], bf16, tag="wv")
        wq_sb = [wq_sb2[:, c * 384:(c + 1) * 384] for c in range(CK)]
        wv_sb = [wv_sb2[:, c * HPC * D:(c + 1) * HPC * D] for c in range(CK)]
        wpA_sb = sb.tile([128, C], bf16, tag="wpA")   # heads 0,1
        # head 2 lives at partitions 64:128 to match aoT_bc's h2 strip
        wpC_sb = sb.tile([128, C], bf16, tag="wpC")

        # ones columns of v_aug (cols 64,129,194 mod 195) come from this
        # blanket fill; the v copies below overwrite the 64-col data slices.
        nc.gpsimd.memset(v_aug[:], 1.0)

        # PE warm-up: ~3.5us of dummy matmuls bridge the initial xt DMA
        # latency so the p-state ramp completes before the first real matmul
        warm = sb.tile([128, 512], bf16, tag="warm")
        nc.vector.memset(warm[:], 0.5)
        with tc.tile_pool(name="wups", bufs=1, space="PSUM") as wu_ps:
            wu = wu_ps.tile([128, 512], f32, tag="wu")
            for _ in range(12):
                nc.tensor.matmul(wu[:], warm[:, 0:128], warm[:],
                                 start=True, stop=True,
                                 skip_group_check=True)

        # xt strips: whole [128, T] rows per contraction chunk; the j=0
        # slice loads first so the pipeline can start early.
        xt_sb2 = sb.tile([128, CK * T], bf16, tag="xts")
        xt_sb = [xt_sb2[:, c * T:(c + 1) * T] for c in range(CK)]
        xt3 = xt_sb2[:].rearrange("p (c t) -> p c t", t=T)
        xt3_d = xt_d.rearrange("p (c t) -> p c t", t=T)

        def xt_wave(a, b):
            nc.sync.dma_start(xt3[:, :, a:b], xt3_d[:, :, a:b])

        nc.sync.dma_start(wq_sb2[:], wq_d)
        xt_wave(0, 512)
        # bq is only needed at the first bias-add, well after these waves;
        # keeping it off the front saves a serial HWDGE slot on the head
        nc.sync.dma_start(bq_sb[:], bq_d)
        xt_wave(512, 1024)
        nc.sync.dma_start(wv_sb2[:], wv_d)
        nc.sync.dma_start(wpA_sb[:], wp_d[0:128, :])
        nc.sync.dma_start(wpC_sb[64:128, :], wp_d[128:192, :])
        for half in range(1, 4):
            xt_wave(half * 1024, (half + 1) * 1024)
        # PSUM budget (8 banks): sc 2x2 + acc 2 + qkv 2. The acc slots
        # also serve the deferred projection (they are idle between norm(j)
        # and attv(j+1)); qkv rotates its 5 generations through 2 banks.
        sc_ps = ctx.enter_context(
            tc.tile_pool(name="scps", bufs=2, space="PSUM"))
        at_ps = ctx.enter_context(
            tc.tile_pool(name="atps", bufs=2, space="PSUM"))
        qk_ps = ctx.enter_context(
            tc.tile_pool(name="qkps", bufs=2, space="PSUM"))
        eb_pool = ctx.enter_context(tc.tile_pool(name="ebp", bufs=12))
        ao_pool = ctx.enter_context(tc.tile_pool(name="aop", bufs=6))
        aoT_pool = ctx.enter_context(tc.tile_pool(name="aotp", bufs=6))
        y_pool = ctx.enter_context(tc.tile_pool(name="yp", bufs=4))
        nrm = ctx.enter_context(tc.tile_pool(name="nrm", bufs=4))

        def qkv_tasks(j):
            """qkv projection for q-tile j as a list of filler closures (one
            per psum generation) so the PE work can interleave between
            attention rounds of the previous tile."""
            jsl = bass.ts(j, 512)
            xt_t = [xt_sb[c][:, jsl] for c in range(CK)]

            def qk_slot(ps_reg, s):
                c0, c1 = QK_SLOTS[s]
                for c in range(CK):
                    nc.tensor.matmul(ps_reg, wq_sb[c][:, c0:c1], xt_t[c],
                                     start=(c == 0), stop=(c == CK - 1),
                                     skip_group_check=True)

            def v_chunk(ps_reg, tc_):
                for c in range(CK):
                    nc.tensor.matmul(ps_reg,
                                     xt_t[c][:, tc_ * 128:(tc_ + 1) * 128],
                                     wv_sb[c],
                                     start=(c == 0), stop=(c == CK - 1),
                                     skip_group_check=True)

            def v_store(ps_reg, tc_):
                base = (4 * j + tc_) * 195
                dst = v_aug[:, base:base + 195].rearrange(
                    "p (h c) -> p h c", c=65)[:, :, 0:64]
                nc.vector.tensor_copy(
                    dst, ps_reg.rearrange("p (h c) -> p h c", c=64))

            def gen1():
                g = qk_ps.tile([128, 512], f32, tag="qk", name="qkv1")
                qk_slot(g[:], 0)
                nc.vector.tensor_scalar_add(qT_AB[:, jsl], g[:],
                                            bq_sb[:, 0:1])

            def gen2():
                g = qk_ps.tile([128, 512], f32, tag="qk", name="qkv2")
                qk_slot(g[:], 1)
                nc.vector.tensor_scalar_add(kT_AB[:, jsl], g[:],
                                            bq_sb[:, 1:2])

            def gen3():
                g = qk_ps.tile([128, 512], f32, tag="qk", name="qkv3")
                qk_slot(g[:], 2)
                # q2 rows 0:64, k2 rows 64:128 stacked in one bank
                nc.vector.tensor_scalar_add(qT_C[0:64, jsl], g[0:64, :],
                                            bq_sb[0:64, 2:3])
                nc.vector.tensor_scalar_add(kT_C[64:128, jsl], g[64:128, :],
                                            bq_sb[64:128, 2:3])
                # duplicate head-2 q/k into the other 64-partition strip so
                # the scores matmul can alternate strips (operands must share
                # the partition range)
                nc.sync.dma_start(qT_C[64:128, jsl], qT_C[0:64, jsl])
                nc.sync.dma_start(kT_C[0:64, jsl], kT_C[64:128, jsl])

            def mkv(tc_):
                # one accumulation group per psum generation: a start=True
                # in a bank wipes other in-flight groups' pending state
                def gen():
                    g = qk_ps.tile([128, 512], f32, tag="qk",
                                   name=f"qkv4_{tc_}")
                    v_chunk(g[:, 0:192], tc_)
                    v_store(g[:, 0:192], tc_)
                return gen

            # (must_before_tile, cost, closure): gen1/2/3 must run early
            # enough that tile j's scores never wait on them; the v chunks
            # by the start of attention(j)
            gens = [gen1, gen2, gen3] + [mkv(t) for t in range(4)]
            tags = [j - 0.4] * 3 + [j] * 4
            costs = [1280] * 3 + [480] * 4
            return list(zip(tags, costs, gens))

        import os
        KSTAGE = int(os.environ.get("KSTAGE", "5"))

        # chunks whose normalize+transpose ran in iteration j; their output
        # projection is deferred into iteration j+1 so the PE never waits on
        # a transpose DMA at the head of its in-order queue.
        pending_proj = []

        def proj_tasks():
            """Deferred output projection of the chunks queued in
            pending_proj, as filler closures. py psum comes from the qk pool
            (its consumers depend only on their own producers, so
            interleaving cannot deadlock the PE FIFO)."""
            if KSTAGE < 5:
                pending_proj.clear()
                return []
            tasks = []
            for m, aoT_ab, aoT_bc in pending_proj:
                def mk(m, aoT_ab, aoT_bc):
                    state = {}

                    def genA():
                        state["y"] = y_pool.tile([128, C], bf16, tag="y",
                                                 name="ysb")
                        py = qk_ps.tile([128, 512], f32, tag="qk",
                                        name="py")[:, 0:384]
                        nc.tensor.matmul(py, aoT_ab[:], wpA_sb[:, 0:384],
                                         start=True, stop=False,
                                         skip_group_check=True)
                        nc.tensor.matmul(py, aoT_bc[64:128, :],
                                         wpC_sb[64:128, 0:384],
                                         start=False, stop=True,
                                         skip_group_check=True)
                        nc.vector.tensor_copy(state["y"][:, 0:384], py)

                    def genB():
                        py = qk_ps.tile([128, 512], f32, tag="qk",
                                        name="py")[:, 0:384]
                        nc.tensor.matmul(py, aoT_ab[:], wpA_sb[:, 384:768],
                                         start=True, stop=False,
                                         skip_group_check=True)
                        nc.tensor.matmul(py, aoT_bc[64:128, :],
                                         wpC_sb[64:128, 384:768],
                                         start=False, stop=True,
                                         skip_group_check=True)
                        nc.vector.tensor_copy(state["y"][:, 384:768], py)
                        nc.sync.dma_start(y_d[m * 128:(m + 1) * 128, :],
                                          state["y"][:])

                    return [(320, genA), (320, genB)]
                tasks.extend(mk(m, aoT_ab, aoT_bc))
            pending_proj.clear()
            return tasks

        def emit_attention(j, filler):
            """scores -> exp -> mask -> att@v -> normalize -> transpose for
            q-tile j (heads 0,1 via AB tiles; head 2 via C). att@v for a
            round is emitted one round late so its exp/mask dependencies are
            already satisfied when the PE reaches it. One filler closure
            (qkv of j+1 / deferred proj of j-1) is emitted before each
            round's scores so the PE has independent work while a score
            matmul waits on its psum slot."""
            nk = 4 * j + 4
            # force any work that must precede this tile (its own qkv gens)
            while filler and filler[0][0] <= j:
                filler.popleft()[2]()
            debt = [0.0]

            def pop_filler(deficit):
                debt[0] += deficit
                while filler and debt[0] >= filler[0][1]:
                    tile_req, cost, fn = filler.popleft()
                    fn()
                    debt[0] -= cost

            # att accumulators: 12 of [128, 65] packed into two banks.
            # Interleaved matmul groups share each bank, so no matmul may use
            # start=True (it marks the whole 2KB zero-region pending and
            # corrupts the other groups); the banks are zeroed by DVE memset
            # instead and every attv accumulates.
            accA = at_ps.tile([128, 512], f32, tag="acc", name="accA")
            accB = at_ps.tile([128, 512], f32, tag="acc", name="accB")
            nc.vector.memset(accA[:, 0:455], 0.0)
            nc.vector.memset(accB[:, 0:325], 0.0)

            def acc_ap(h, qc):
                i = h * 4 + qc
                if i < 7:
                    return accA[:, i * 65:(i + 1) * 65]
                return accB[:, (i - 7) * 65:(i - 6) * 65]

            def attv(eb, bank, ki, h):
                if KSTAGE < 3:
                    return
                r = ki - 4 * j
                for qc in range(max(r, 0), 4):
                    nc.tensor.matmul(
                        acc_ap(h, qc),
                        eb[:, bank * 512 + qc * 128:bank * 512 + qc * 128 + 128],
                        v_aug[:, ki * 195 + 65 * h:ki * 195 + 65 * h + 65],
                        start=False, stop=(ki == 4 * j + qc),
                        skip_group_check=True)

            def exp_mask(pr, eb, bank, ki, single=True):
                """exp (trimmed to the valid q-range) + causal mask for one
                512-col bank; with single=False the caller batches the exp."""
                r = ki - 4 * j
                t0 = 128 * max(r, 0)
                if single:
                    nc.scalar.activation(
                        eb[:, bank * 512 + t0:(bank + 1) * 512],
                        pr[:, bank * 512 + t0:(bank + 1) * 512],
                        AF.Exp, scale=0.125)
                if r >= 0:
                    # only the exact-diagonal 128-col block needs masking;
                    # everything past it is strictly below the diagonal
                    sl = eb[:, bank * 512 + t0:bank * 512 + t0 + 128]
                    nc.gpsimd.affine_select(
                        sl, sl, pattern=[[1, 128]],
                        compare_op=mybir.AluOpType.is_ge, fill=0.0,
                        base=0, channel_multiplier=-1)

            # --- heads 0,1: one ki per round, 2 psum banks ---
            from collections import deque as _dq
            pend = _dq()  # (eb, ki) of recent rounds; attv runs 2 late
            for ki in range(nk):
                r = ki - 4 * j  # >=0 on the diagonal band
                t0 = 128 * max(r, 0)  # valid q-range start within the tile
                act = 2 * (512 - t0) * 0.833 + (370 if t0 else 185)
                pe = 2 * (512 - t0) * 0.417 + 240
                pop_filler((act - pe) * 0.55)
                ksl = bass.ts(ki, 128)
                pr = sc_ps.tile([128, 1024], f32, tag="sc", name="sc")
                for hh in (0, 1):
                    r0 = 64 * hh
                    nc.tensor.matmul(
                        pr[:, hh * 512 + t0:(hh + 1) * 512],
                        kT_AB[r0:r0 + 64, ksl],
                        qT_AB[r0:r0 + 64, j * 512 + t0:(j + 1) * 512],
                        start=True, stop=True)
                eb = eb_pool.tile([128, 1024], bf16, tag="eb", name="eb")
                if t0 == 0:
                    nc.scalar.activation(eb[:], pr[:], AF.Exp, scale=0.125)
                else:
                    src3 = pr[:].rearrange("p (b q) -> p b q",
                                           q=512)[:, :, t0:512]
                    dst3 = eb[:].rearrange("p (b q) -> p b q",
                                           q=512)[:, :, t0:512]
                    nc.scalar.activation(dst3, src3, AF.Exp, scale=0.125)
                exp_mask(pr, eb, 0, ki, single=False)
                exp_mask(pr, eb, 1, ki, single=False)
                if dbg and j == 0 and ki == int(__import__("os").environ.get("KEB", "0")):
                    nc.sync.dma_start(dbg_out["d_eb"], eb[:])
                pend.append((eb, ki))
                if len(pend) > 2:
                    peb, pki = pend.popleft()
                    attv(peb, 0, pki, 0)
                    attv(peb, 1, pki, 1)
            while pend:
                peb, pki = pend.popleft()
                attv(peb, 0, pki, 0)
                attv(peb, 1, pki, 1)

            # --- head 2: two ki per round using the duplicated C strips ---
            pend = _dq()
            while filler and filler[0][0] <= j + 0.6:
                filler.popleft()[2]()
            for g0 in range(0, nk, 2):
                pop_filler(100)
                pr = sc_ps.tile([128, 1024], f32, tag="sc", name="sc")
                eb = eb_pool.tile([128, 1024], bf16, tag="eb", name="eb")
                diag = g0 + 1 >= 4 * j
                for idx, ki in enumerate((g0, g0 + 1)):
                    r = ki - 4 * j
                    t0 = 128 * max(r, 0)
                    ksl = bass.ts(ki, 128)
                    r0 = 64 * (idx % 2)
                    nc.tensor.matmul(
                        pr[:, idx * 512 + t0:(idx + 1) * 512],
                        kT_C[r0:r0 + 64, ksl],
                        qT_C[r0:r0 + 64, j * 512 + t0:(j + 1) * 512],
                        start=True, stop=True)
                if diag:
                    for idx, ki in enumerate((g0, g0 + 1)):
                        exp_mask(pr, eb, idx, ki)
                else:
                    nc.scalar.activation(eb[:], pr[:], AF.Exp, scale=0.125)
                pend.append((eb, g0))
                if len(pend) > 2:
                    peb, pg0 = pend.popleft()
                    attv(peb, 0, pg0, 2)
                    attv(peb, 1, pg0 + 1, 2)
            while pend:
                peb, pg0 = pend.popleft()
                attv(peb, 0, pg0, 2)
                attv(peb, 1, pg0 + 1, 2)

            # --- normalize + transpose per 128-chunk; projection deferred.
            # Raw psum->sbuf copies release the acc banks quickly so
            # attv(j+1) is not gated on the full normalization chain. ---
            accs = nrm.tile([128, 780], f32, tag="accs", name="accs")
            nc.vector.tensor_copy(accs[:, 0:455], accA[:, 0:455])
            nc.vector.tensor_copy(accs[:, 455:780], accB[:, 0:325])

            def acc_sb(h, qc):
                i = h * 4 + qc
                return accs[:, i * 65:(i + 1) * 65]

            if dbg and j == 0:
                nc.sync.dma_start(dbg_out["d_accs"], accs[:])

            # all 12 denominators inverted in ONE strided custom-DVE op
            # (they sit at accs cols 64+65*i) -- keeps the tile-end DVE
            # chain short
            rcp12 = nrm.tile([128, 12], f32, tag="rcp", name="rcp12")
            if KSTAGE >= 4:
                den = accs[:].rearrange("p (i c) -> p i c", c=65)[:, :, 64:65]
                nc.vector.reciprocal_approx_fast(
                    out=rcp12[:].rearrange("p (i c) -> p i c", c=1),
                    in_=den)
            for qc in range(4 if KSTAGE >= 4 else 0):
                pop_filler(400)
                m = 4 * j + qc
                ao = ao_pool.tile([128, HPC * D], bf16, tag="ao", name="ao")
                for h in range(HPC):
                    a = acc_sb(h, qc)
                    i = h * 4 + qc
                    nc.vector.tensor_scalar_mul(
                        ao[:, h * 64:(h + 1) * 64], a[:, 0:64],
                        rcp12[:, i:i + 1])
                aoT_ab = aoT_pool.tile([128, 128], bf16, tag="tab", name="tab")
                aoT_bc = aoT_pool.tile([128, 128], bf16, tag="tbc", name="tbc")
                nc.sync.dma_start_transpose(aoT_ab[:], ao[:, 0:128])
                nc.sync.dma_start_transpose(aoT_bc[:], ao[:, 64:192])
                if dbg and j == 0 and qc == 0:
                    nc.sync.dma_start(dbg_out["d_ao"], ao[:])
                    nc.sync.dma_start(dbg_out["d_aotab"], aoT_ab[:])
                    nc.sync.dma_start(dbg_out["d_aotbc"], aoT_bc[:])
                pending_proj.append((m, aoT_ab, aoT_bc))
                if j == NT - 1 and KSTAGE >= 5:
                    # last tile: flush inline on the now-idle sc banks (two
                    # independent banks per chunk) so the projection chain
                    # does not serialize through the 2-slot qk pool
                    for m2, tab, tbc in pending_proj:
                        y_sb = y_pool.tile([128, C], bf16, tag="y",
                                           name="ysb2")
                        pw = sc_ps.tile([128, 1024], f32, tag="sc",
                                        name="pyw")
                        for ns in range(2):
                            py = pw[:, ns * 512:ns * 512 + 384]
                            nc.tensor.matmul(py, tab[:],
                                             wpA_sb[:, ns * 384:(ns + 1) * 384],
                                             start=True, stop=False,
                                             skip_group_check=True)
                            nc.tensor.matmul(py, tbc[64:128, :],
                                             wpC_sb[64:128,
                                                    ns * 384:(ns + 1) * 384],
                                             start=False, stop=True,
                                             skip_group_check=True)
                            nc.vector.tensor_copy(
                                y_sb[:, ns * 384:(ns + 1) * 384], py)
                        nc.sync.dma_start(y_d[m2 * 128:(m2 + 1) * 128, :],
                                          y_sb[:])
                    pending_proj.clear()

        from collections import deque
        filler = deque()  # entries: (must_before_tile, cost_ns, closure)
        for _, _, gen in qkv_tasks(0):
            gen()
        # tile 1's q/k generations run up front too: tile 0's attention is
        # too small to host them as filler without starving the ACT engine
        t1 = qkv_tasks(1)
        for _, _, gen in t1[:3]:
            gen()
        for j in range(NT):
            if j == 0:
                filler.extend(t1[3:])
            elif j + 1 < NT:
                filler.extend(qkv_tasks(j + 1))
            filler.extend((j + 2, c, fn) for c, fn in proj_tasks())
            if KSTAGE >= 2:
                emit_attention(j, filler)
            else:
                while filler:
                    filler.popleft()[2]()
        while filler:
            filler.popleft()[2]()
        for _, t in proj_tasks():
            t()
        if dbg:
            nc.sync.dma_start(dbg_out["d_qAB"], qT_AB[:])
            nc.sync.dma_start(dbg_out["d_kAB"], kT_AB[:])
            nc.sync.dma_start(dbg_out["d_qC"], qT_C[:])
            nc.sync.dma_start(dbg_out["d_kC"], kT_C[:])
            nc.sync.dma_start(dbg_out["d_vaug"], v_aug[:])

    nc.compile()
    return nc


_NC_CACHE = {}


def _get_nc(T):
    if T not in _NC_CACHE:
        _NC_CACHE[T] = build_nc(T)
    return _NC_CACHE[T]


def make_core_inputs(x, W_attn, b_attn, W_proj):
    """Host-side prep: per-core input dicts (see module docstring)."""
    B, T, _ = x.shape
    xts = [np.ascontiguousarray(x[b].T) for b in range(B)]
    in_maps = []
    for core in range(N_CORES):
        b = core // (N_CORES // B)
        h0 = HPC * (core % (N_CORES // B))
        ccols = slice(h0 * D, (h0 + 2) * D)      # first two heads
        c2 = slice((h0 + 2) * D, (h0 + 3) * D)   # third head
        # reference splits qkv as (k, q, v): k cols 0:C, q cols C:2C, v 2C:3C
        q01 = W_attn[:, C:2 * C][:, ccols]
        k01 = W_attn[:, 0:C][:, ccols]
        q2 = W_attn[:, C:2 * C][:, c2]
        k2 = W_attn[:, 0:C][:, c2]
        wq = np.ascontiguousarray(
            np.concatenate([q01, k01, q2, k2], axis=1))
        wv = np.ascontiguousarray(
            W_attn[:, 2 * C:3 * C][:, h0 * D:(h0 + HPC) * D])
        bq = np.zeros((128, 3), np.float32)
        bq[:, 0] = b_attn[C:2 * C][ccols]
        bq[:, 1] = b_attn[0:C][ccols]
        bq[0:64, 2] = b_attn[C:2 * C][c2]
        bq[64:128, 2] = b_attn[0:C][c2]
        wp = np.ascontiguousarray(W_proj[h0 * D:(h0 + HPC) * D, :])
        def pmaj(a):
            # [CK*128, f] -> [128, CK*f]
            f = a.shape[1]
            return np.ascontiguousarray(
                a.reshape(-1, 128, f).transpose(1, 0, 2).reshape(128, -1))

        in_maps.append({
            "xt": to_bf16(pmaj(xts[b])),
            "wq": to_bf16(pmaj(wq)),
            "wv": to_bf16(pmaj(wv)),
            "bq": bq,
            "wp": to_bf16(wp),
        })
    return in_maps


def to_bf16(a):
    import ml_dtypes
    return np.ascontiguousarray(a.astype(ml_dtypes.bfloat16))


def kernel(x, W_attn, b_attn, W_proj, b_proj):
    x = np.asarray(x, dtype=np.float32)
    W_attn = np.asarray(W_attn, dtype=np.float32)
    b_attn = np.asarray(b_attn, dtype=np.float32)
    W_proj = np.asarray(W_proj, dtype=np.float32)
    b_proj = np.asarray(b_proj, dtype=np.float32)
    B, T, _ = x.shape

    nc = _get_nc(T)
    in_maps = make_core_inputs(x, W_attn, b_attn, W_proj)
    res = None
    for attempt in range(3):
        try:
            res = run_bass_kernel_spmd(nc, in_maps, list(range(N_CORES)))
            break
        except Exception:
            # transient NRT_EXEC_UNIT_UNRECOVERABLE has been observed once
            # after a prior crashed process; a retry succeeds
            if attempt == 2:
                raise
    global LAST_RUN
    LAST_RUN = res

    # the v bias contributes b_v @ W_proj to every token (softmax rows sum
    # to 1), folded here instead of inside the kernel
    b_eff = b_proj + b_attn[2 * C:3 * C] @ W_proj

    gpb = N_CORES // B
    out = np.empty((B, T, C), np.float32)
    for b in range(B):
        acc = res.results[b * gpb]["y"].astype(np.float32)
        for g in range(1, gpb):
            acc = acc + res.results[b * gpb + g]["y"]
        out[b] = acc + b_eff[None, :]
    return out
